# revision 1
# baseline (speedup 1.0000x reference)
"""ChebNet (2-layer ChebConv, K=3) on 8 Trainium2 NeuronCores.

Strategy
--------
Math: propagation commutes with the per-order weight matmul, so the two
ChebConv layers reduce to 4 sparse propagations on raw features plus tiny
dense matmuls:
    L1: out1 = x(W10-W12) + Tx1*W11 + 2*Tx2*W12 + b1,  Tx1 = L x, Tx2 = L Tx1
    h = relu(out1)
    L2: out2 = h(W20-W22) + U1*W21 + 2*U2*W22 + b2,    U1 = L h, U2 = L U1
where L[c,r] = sum over edges (r->c) of -dinv[r]*w*dinv[c]  (PyG ChebConv
normalization with lambda_max=2).

Sharding: each of the 8 cores owns edges with source in one of 4 contiguous
25024-row windows and dest in one of 2 halves (4 chunks x 2 halves). Each
core gathers source rows from its window (int16 dma_gather indices), forms
per-edge messages, and aggregates them into its half's destination tiles
with an is_equal-selector matmul on the tensor engine. Host reduces the 4
partial aggregates per half between launches (pure data movement + adds).

Device pipeline per pass: dma_gather (1024 rows/call) -> DVE builds
S[t,d] = norm[t] * (iota[d] == local_dest[t]) -> PE matmul accumulates
psum[d, :] += S^T @ messages -> psum copied out per 64-node dest tile.
"""
import numpy as np
from contextlib import ExitStack

import concourse.bass as bass
import concourse.bacc as bacc
import concourse.mybir as mybir
import concourse.tile as tile
from concourse.bass_utils import run_bass_kernel_spmd

# problem constants (hardcoded per harness contract)
N = 100000
E = 1600000
F_IN = 128
F_HID = 64
F_OUT = 40
K = 3

P = 128
D = 64                 # dest-tile width (nodes per psum tile)
NPAD = 100096          # padded node count: /128 = 782, /64 = 1564
NCHUNK = 4
CH = NPAD // NCHUNK    # 25024 source rows per chunk (< 32768 for int16 idx)
NHALF = 2
HALF = NPAD // NHALF   # 50048 dest rows per half
TS = HALF // D         # 782 dest tiles per half
NCORES = 8
NS = NPAD // NCORES    # 12512 nodes per core for dense epilogues
CALL_BLOCKS = 8        # 1024 gather rows per dma_gather (descriptor ring limit)

_DT = mybir.dt.float32


# ---------------------------------------------------------------------------
# host-side graph preprocessing
# ---------------------------------------------------------------------------

def _prep_graph(edge_index, edge_weight):
    """Partition edges into 8 (chunk, half) cores; build per-core slot arrays.

    Returns dict with per-core int16 gather indices (wrapped layout), meta
    (ld/norm) arrays, the shared block structure NB[slot->nblocks], call plan,
    and per-core tile-id permutations.
    """
    row = np.ascontiguousarray(edge_index[0]).astype(np.int64)
    col = np.ascontiguousarray(edge_index[1]).astype(np.int64)
    w = np.ascontiguousarray(edge_weight).astype(np.float32)

    deg = np.bincount(row, weights=w.astype(np.float64), minlength=N).astype(np.float32)
    dinv = np.where(deg > 0, 1.0 / np.sqrt(np.maximum(deg, 1e-30)), 0.0).astype(np.float32)
    norm = (-dinv[row] * w * dinv[col]).astype(np.float32)

    chunk = row // CH
    half = col // HALF
    core_of_edge = (half * NCHUNK + chunk).astype(np.int64)

    # per (core, tile) edge counts; tile id local to the half
    ltile = (col % HALF) // D
    ld = (col % HALF) % D

    cores = []
    nblocks_sorted = []
    for c in range(NCORES):
        sel = np.nonzero(core_of_edge == c)[0]
        # order edges by local tile for contiguous tile runs
        order = np.argsort(ltile[sel], kind="stable")
        sel = sel[order]
        t_of_e = ltile[sel]
        counts = np.bincount(t_of_e, minlength=TS)
        nb = np.maximum(1, -(-counts // P))  # ceil, min 1 block per tile
        # sort tiles by descending block count, stable by tile id
        perm = np.lexsort((np.arange(TS), -nb))
        cores.append(dict(sel=sel, counts=counts, nb=nb, perm=perm))
        nblocks_sorted.append(nb[perm])

    NB = np.max(np.stack(nblocks_sorted), axis=0)  # shared per-slot block count
    B_TOTAL = int(NB.sum())
    SLOTS = B_TOTAL * P

    # call plan: runs of <= CALL_BLOCKS blocks (identical for all cores)
    calls = []
    b = 0
    while b < B_TOTAL:
        n = min(CALL_BLOCKS, B_TOTAL - b)
        calls.append((b, n))
        b += n

    # per-core slot arrays; pad slots gather row 0 (zeroed by ld=-1 in S)
    idx16 = np.zeros((NCORES, SLOTS), np.int16)
    ld_f = np.full((NCORES, SLOTS), -1.0, np.float32)
    nrm_f = np.zeros((NCORES, SLOTS), np.float32)
    tile_ids = np.zeros((NCORES, TS), np.int64)

    slot_tile_start = np.concatenate([[0], np.cumsum(NB)]) * P  # per sorted slot
    for c in range(NCORES):
        st = cores[c]
        sel, counts, perm = st["sel"], st["counts"], st["perm"]
        tile_ids[c] = perm
        # edge offsets per tile in the tile-ordered edge list
        e_start = np.concatenate([[0], np.cumsum(counts)])
        ch_base = (c % NCHUNK) * CH
        for s in range(TS):
            t = perm[s]
            cnt = counts[t]
            if cnt == 0:
                continue
            eids = sel[e_start[t] : e_start[t] + cnt]
            base = slot_tile_start[s]
            idx16[c, base : base + cnt] = (row[eids] - ch_base).astype(np.int16)
            ld_f[c, base : base + cnt] = ld[eids].astype(np.float32)
            nrm_f[c, base : base + cnt] = norm[eids]

    # wrap idx to dma_gather layout [16, SLOTS/16] at [i%16, i//16], tile 8x
    ii = np.arange(SLOTS)
    idxw = np.zeros((NCORES, 16, SLOTS // 16), np.int16)
    idxw[:, ii % 16, ii // 16] = idx16
    idxw = np.tile(idxw, (1, 8, 1))  # [NCORES, 128, SLOTS/16]

    # meta [128, D + 2*B_TOTAL]: iota then per-block (ld, norm) column pairs.
    # slot i of call k maps to partition i%128, block (global) i//128.
    meta = np.zeros((NCORES, P, D + 2 * B_TOTAL), np.float32)
    meta[:, :, :D] = np.arange(D, dtype=np.float32)[None, None, :]
    ld_pb = ld_f.reshape(NCORES, B_TOTAL, P).transpose(0, 2, 1)   # [C, 128, B]
    nrm_pb = nrm_f.reshape(NCORES, B_TOTAL, P).transpose(0, 2, 1)
    meta[:, :, D::2] = ld_pb
    meta[:, :, D + 1 :: 2] = nrm_pb

    return dict(
        NB=NB, B_TOTAL=B_TOTAL, SLOTS=SLOTS, calls=calls,
        idxw=idxw, meta=meta, tile_ids=tile_ids, dinv=dinv,
    )


# ---------------------------------------------------------------------------
# device program builders
# ---------------------------------------------------------------------------

def _build_pass_program(F, NB, calls, B_TOTAL, SLOTS, dense=None):
    """One propagation pass: gather + selector-matmul aggregation.

    Inputs per core: srcw [CH, F], idx [128, SLOTS/16] i16,
    meta [128, D+2*B_TOTAL] f32.
    Output: part [TS, D, F] f32 (per-slot aggregates; host unpermutes).

    dense: None or dict(K=contract dim, FO=out feats) adding the Z epilogue:
      extra inputs aT [K, NS], bT [K, NS], wz [K, 2*FO], bz [1, FO]
      extra output z [NS, FO] with z = a^T rows @ wz[:, :FO] + b^T rows @
      wz[:, FO:] + bz  (per 128-row chunks).
    """
    nc = bacc.Bacc("TRN2", target_bir_lowering=False)
    srcw = nc.declare_dram_parameter("srcw", [CH, F], _DT, isOutput=False)
    idx = nc.declare_dram_parameter("idx", [P, SLOTS // 16], mybir.dt.int16, isOutput=False)
    meta = nc.declare_dram_parameter("meta", [P, D + 2 * B_TOTAL], _DT, isOutput=False)
    # part laid out [D, TS, F] so grouped tile writes are contiguous 4KB runs
    part = nc.declare_dram_parameter("part", [D, TS, F], _DT, isOutput=True)
    if dense is not None:
        KD, FO = dense["K"], dense["FO"]
        aT = nc.declare_dram_parameter("aT", [KD, NS], _DT, isOutput=False)
        bT = nc.declare_dram_parameter("bT", [KD, NS], _DT, isOutput=False)
        wz = nc.declare_dram_parameter("wz", [KD, 2 * FO], _DT, isOutput=False)
        bz = nc.declare_dram_parameter("bz", [1, FO], _DT, isOutput=False)
        z = nc.declare_dram_parameter("z", [NS, FO], _DT, isOutput=True)

    # slot -> tile boundaries
    tile_of_block = np.repeat(np.arange(len(NB)), NB)
    first_block = np.concatenate([[0], np.cumsum(NB)[:-1]])
    last_block = np.cumsum(NB) - 1
    OG = 8  # output tiles per grouped DRAM write

    with ExitStack() as ctx:
        tc = ctx.enter_context(tile.TileContext(nc))
        cpool = ctx.enter_context(tc.tile_pool(name="const", bufs=1))
        gpool = ctx.enter_context(tc.tile_pool(name="g", bufs=4))
        spool = ctx.enter_context(tc.tile_pool(name="s", bufs=8))
        opool = ctx.enter_context(tc.tile_pool(name="o", bufs=3))
        ppool = ctx.enter_context(tc.tile_pool(name="ps", bufs=4, space="PSUM"))

        idx_t = cpool.tile([P, SLOTS // 16], mybir.dt.int16)
        meta_t = cpool.tile([P, D + 2 * B_TOTAL], _DT)
        nc.sync.dma_start(out=idx_t[:], in_=idx[:])
        nc.sync.dma_start(out=meta_t[:], in_=meta[:])
        iota_f = meta_t[:, 0:D]

        psum = None
        cur_tile = -1
        for (b0, nb) in calls:
            g = gpool.tile([P, nb, F], _DT, tag="g")
            nc.gpsimd.dma_gather(
                g[:],
                srcw[:],
                idx_t[:, b0 * 8 : (b0 + nb) * 8],
                nb * P, nb * P, F,
            )
            for j in range(nb):
                blk = b0 + j
                t = int(tile_of_block[blk])
                S = spool.tile([P, D], _DT, tag="S")
                mc = D + 2 * blk
                nc.vector.tensor_scalar(
                    out=S[:], in0=iota_f,
                    scalar1=meta_t[:, mc : mc + 1],
                    scalar2=meta_t[:, mc + 1 : mc + 2],
                    op0=mybir.AluOpType.is_equal,
                    op1=mybir.AluOpType.mult,
                )
                if blk == first_block[t]:
                    psum = ppool.tile([D, F], _DT, space="PSUM", tag="acc")
                nc.tensor.matmul(
                    out=psum[:], lhsT=S[:], rhs=g[:, j, :],
                    start=(blk == first_block[t]),
                    stop=(blk == last_block[t]),
                )
                if blk == last_block[t]:
                    gi, go = t // OG, t % OG
                    if go == 0:
                        og = opool.tile([D, OG, F], _DT, tag="o")
                    nc.vector.tensor_copy(og[:, go, :], psum[:])
                    if go == OG - 1 or t == len(NB) - 1:
                        nc.sync.dma_start(
                            out=part[:, gi * OG : gi * OG + go + 1, :],
                            in_=og[:, : go + 1, :],
                        )

        if dense is not None:
            dpool = ctx.enter_context(tc.tile_pool(name="dz", bufs=3))
            zpool = ctx.enter_context(tc.tile_pool(name="zz", bufs=3))
            zps = ctx.enter_context(tc.tile_pool(name="zps", bufs=2, space="PSUM"))
            wz_t = cpool.tile([KD, 2 * FO], _DT)
            bz_t = cpool.tile([1, FO], _DT)
            ones_t = cpool.tile([1, P], _DT)
            nc.sync.dma_start(out=wz_t[:], in_=wz[:])
            nc.sync.dma_start(out=bz_t[:], in_=bz[:])
            nc.vector.memset(ones_t[:], 1.0)
            SC = 1024  # super-chunk columns per load
            nsup = -(-NS // SC)
            for sj in range(nsup):
                sc = min(SC, NS - sj * SC)
                a_t = dpool.tile([KD, SC], _DT, tag="a")
                b_t = dpool.tile([KD, SC], _DT, tag="b")
                nc.sync.dma_start(out=a_t[:, :sc], in_=aT[:, sj * SC : sj * SC + sc])
                nc.sync.dma_start(out=b_t[:, :sc], in_=bT[:, sj * SC : sj * SC + sc])
                zo = zpool.tile([P, SC // P, FO], _DT, tag="zo")
                nj = -(-sc // P)
                for j in range(nj):
                    m = min(P, sc - j * P)
                    pz = zps.tile([P, FO], _DT, space="PSUM", tag="z")
                    nc.tensor.matmul(out=pz[:m], lhsT=a_t[:, j * P : j * P + m],
                                     rhs=wz_t[:, :FO], start=True, stop=False)
                    nc.tensor.matmul(out=pz[:m], lhsT=b_t[:, j * P : j * P + m],
                                     rhs=wz_t[:, FO:], start=False, stop=False)
                    nc.tensor.matmul(out=pz[:m], lhsT=ones_t[:, :m], rhs=bz_t[:],
                                     start=False, stop=True)
                    nc.vector.tensor_copy(zo[:m, j, :], pz[:m])
                if sc == SC:
                    nc.sync.dma_start(
                        out=z[sj * SC : sj * SC + sc].rearrange("(j p) f -> p j f", p=P),
                        in_=zo[:, :nj, :],
                    )
                else:
                    for j in range(nj):
                        m = min(P, sc - j * P)
                        nc.sync.dma_start(
                            out=z[sj * SC + j * P : sj * SC + j * P + m],
                            in_=zo[:m, j, :],
                        )

    nc.compile()
    return nc


def _build_combine_program(F, FO, relu, scale2):
    """D launch: combine Z with the reduced propagation result, matmul W.

    Inputs per core: zin [NS, FO], pT [F, NS] (host-reduced, feature-major),
    w [F, FO].  Output: out [NS, FO] = act(zin + scale2 * pT^T @ w).
    """
    nc = bacc.Bacc("TRN2", target_bir_lowering=False)
    zin = nc.declare_dram_parameter("zin", [NS, FO], _DT, isOutput=False)
    pT = nc.declare_dram_parameter("pT", [F, NS], _DT, isOutput=False)
    w = nc.declare_dram_parameter("w", [F, FO], _DT, isOutput=False)
    out = nc.declare_dram_parameter("out", [NS, FO], _DT, isOutput=True)

    with ExitStack() as ctx:
        tc = ctx.enter_context(tile.TileContext(nc))
        cpool = ctx.enter_context(tc.tile_pool(name="const", bufs=1))
        dpool = ctx.enter_context(tc.tile_pool(name="d", bufs=3))
        opool = ctx.enter_context(tc.tile_pool(name="o", bufs=3))
        ppool = ctx.enter_context(tc.tile_pool(name="ps", bufs=4, space="PSUM"))

        w_t = cpool.tile([F, FO], _DT)
        nc.sync.dma_start(out=w_t[:], in_=w[:])

        SC = 1024
        nsup = -(-NS // SC)
        for sj in range(nsup):
            sc = min(SC, NS - sj * SC)
            nj = -(-sc // P)
            pt = dpool.tile([F, SC], _DT, tag="pt")
            zt = dpool.tile([P, SC // P, FO], _DT, tag="zt")
            nc.sync.dma_start(out=pt[:, :sc], in_=pT[:, sj * SC : sj * SC + sc])
            if sc == SC:
                nc.sync.dma_start(
                    out=zt[:],
                    in_=zin[sj * SC : (sj + 1) * SC].rearrange("(j p) f -> p j f", p=P),
                )
            else:
                for j in range(nj):
                    m = min(P, sc - j * P)
                    nc.sync.dma_start(out=zt[:m, j, :],
                                      in_=zin[sj * SC + j * P : sj * SC + j * P + m])
            oo = opool.tile([P, SC // P, FO], _DT, tag="oo")
            for j in range(nj):
                m = min(P, sc - j * P)
                pz = ppool.tile([P, FO], _DT, space="PSUM", tag="z")
                nc.tensor.matmul(out=pz[:m], lhsT=pt[:, j * P : j * P + m],
                                 rhs=w_t[:], start=True, stop=True)
                # oo = zin + scale2 * psum
                nc.vector.tensor_scalar(
                    out=oo[:m, j, :], in0=pz[:m],
                    scalar1=float(scale2), scalar2=None,
                    op0=mybir.AluOpType.mult,
                )
                nc.vector.tensor_add(out=oo[:m, j, :], in0=oo[:m, j, :], in1=zt[:m, j, :])
                if relu:
                    nc.scalar.activation(oo[:m, j, :], oo[:m, j, :],
                                         mybir.ActivationFunctionType.Relu)
            if sc == SC:
                nc.sync.dma_start(
                    out=out[sj * SC : (sj + 1) * SC].rearrange("(j p) f -> p j f", p=P),
                    in_=oo[:],
                )
            else:
                for j in range(nj):
                    m = min(P, sc - j * P)
                    nc.sync.dma_start(out=out[sj * SC + j * P : sj * SC + j * P + m],
                                      in_=oo[:m, j, :])

    nc.compile()
    return nc


# ---------------------------------------------------------------------------
# host glue
# ---------------------------------------------------------------------------

def _pad_rows(a, rows):
    out = np.zeros((rows, a.shape[1]), np.float32)
    out[: a.shape[0]] = a
    return out


def _reduce_partials(parts, tile_ids):
    """parts: list of 8 arrays [D, TS, F] in per-core slot order.
    Returns full [NPAD, F] (sum of the 4 chunk-partials per half)."""
    F = parts[0].shape[2]
    full = np.zeros((NPAD, F), np.float32)
    for c in range(NCORES):
        half = c // NCHUNK
        un = np.zeros((TS, D, F), np.float32)
        un[tile_ids[c]] = parts[c].transpose(1, 0, 2)
        full[half * HALF : (half + 1) * HALF] += un.reshape(HALF, F)
    return full


def _run(nc, in_maps):
    res = run_bass_kernel_spmd(nc, in_maps, list(range(NCORES)))
    return res.results


class _Programs:
    """Compiled program cache for one graph structure."""

    def __init__(self, g):
        self.g = g
        NB, calls, BT, SL = g["NB"], g["calls"], g["B_TOTAL"], g["SLOTS"]
        self.pA = _build_pass_program(F_IN, NB, calls, BT, SL,
                                      dense=dict(K=F_IN, FO=F_HID))
        self.pB = _build_pass_program(F_HID, NB, calls, BT, SL,
                                      dense=dict(K=F_HID, FO=F_OUT))
        self.d1 = _build_combine_program(F_IN, F_HID, relu=True, scale2=2.0)
        self.d2 = _build_combine_program(F_HID, F_OUT, relu=False, scale2=2.0)


def _pass_inputs(g, src_full, zin_a=None, zin_b=None, wz=None, bzv=None, F=None, KD=None, FO=None):
    """Build per-core in_maps for a pass program."""
    maps = []
    for c in range(NCORES):
        chunk = c % NCHUNK
        m = {
            "srcw": src_full[chunk * CH : (chunk + 1) * CH],
            "idx": g["idxw"][c],
            "meta": g["meta"][c],
        }
        if wz is not None:
            sl = slice(c * NS, (c + 1) * NS)
            m["aT"] = np.ascontiguousarray(zin_a[sl].T) if zin_a is not None else np.zeros((KD, NS), np.float32)
            m["bT"] = np.ascontiguousarray(zin_b[sl].T) if zin_b is not None else np.zeros((KD, NS), np.float32)
            m["wz"] = wz
            m["bz"] = bzv.reshape(1, -1)
        maps.append(m)
    return maps


def _combine_inputs(zs, reduced_full, w):
    """zs: [NPAD, FO] z rows; reduced_full: [NPAD, F] reduced propagation."""
    maps = []
    for c in range(NCORES):
        sl = slice(c * NS, (c + 1) * NS)
        maps.append({
            "zin": zs[sl],
            "pT": np.ascontiguousarray(reduced_full[sl].T),
            "w": w,
        })
    return maps


def kernel(x, edge_index, edge_weight, W1, b1, W2, b2):
    x = np.asarray(x, np.float32)
    edge_index = np.asarray(edge_index)
    edge_weight = np.asarray(edge_weight, np.float32)
    W1 = np.asarray(W1, np.float32)
    b1 = np.asarray(b1, np.float32)
    W2 = np.asarray(W2, np.float32)
    b2 = np.asarray(b2, np.float32)

    g = _prep_graph(edge_index, edge_weight)
    progs = _Programs(g)

    xpad = _pad_rows(x, NPAD)

    # dense weight combos
    w1z = np.concatenate([W1[0] - W1[2], W1[1]], axis=1)  # [128, 128]
    w2z = np.concatenate([W2[0] - W2[2], W2[1]], axis=1)  # [64, 80]

    # P1: Tx1 partials (z inputs zero, z output ignored)
    maps = _pass_inputs(g, xpad, None, None, w1z, b1, F=F_IN, KD=F_IN, FO=F_HID)
    res = _run(progs.pA, maps)
    tx1 = _reduce_partials([r["part"] for r in res], g["tile_ids"])

    # P2: Tx2 partials + Z1 = x(W10-W12) + Tx1 W11 + b1
    maps = _pass_inputs(g, tx1, xpad, tx1, w1z, b1, F=F_IN, KD=F_IN, FO=F_HID)
    res = _run(progs.pA, maps)
    tx2 = _reduce_partials([r["part"] for r in res], g["tile_ids"])
    z1 = np.concatenate([r["z"] for r in res], axis=0)  # [NPAD, F_HID]

    # D1: h = relu(Z1 + 2 * Tx2 * W12)
    maps = _combine_inputs(z1, tx2, W1[2])
    res = _run(progs.d1, maps)
    h = np.concatenate([r["out"] for r in res], axis=0)  # [NPAD, F_HID]

    # P3: U1 partials
    maps = _pass_inputs(g, h, None, None, w2z, b2, F=F_HID, KD=F_HID, FO=F_OUT)
    res = _run(progs.pB, maps)
    u1 = _reduce_partials([r["part"] for r in res], g["tile_ids"])

    # P4: U2 partials + Z2 = h(W20-W22) + U1 W21 + b2
    maps = _pass_inputs(g, u1, h, u1, w2z, b2, F=F_HID, KD=F_HID, FO=F_OUT)
    res = _run(progs.pB, maps)
    u2 = _reduce_partials([r["part"] for r in res], g["tile_ids"])
    z2 = np.concatenate([r["z"] for r in res], axis=0)

    # D2: out = Z2 + 2 * U2 * W22
    maps = _combine_inputs(z2, u2, W2[2])
    res = _run(progs.d2, maps)
    out = np.concatenate([r["out"] for r in res], axis=0)

    return out[:N]



# revision 2
# speedup vs baseline: 1.1097x; 1.1097x over previous
"""ChebNet (2-layer ChebConv, K=3) on 8 Trainium2 NeuronCores — v3.

Math: propagation commutes with feature matmuls, so per layer
    out = x(W0-W2) + (Lx)W1 + 2 L((Lx)W2) + b
giving 4 sparse propagations total (2 per layer) plus small dense matmuls
that are fused into the PSUM accumulation of each propagation pass:
  P1: agg1 = L x           -> per tile: [y1 | t11] = agg1^T [2W12 | W11] (+b1)
  P2: h = relu(L y1 + x(W10-W12) + t11)          (all summed in PSUM)
  P3: agg3 = L h           -> per tile: [y2 | u21] = agg3^T [2W22 | W21] (+b2)
  P4: out = L y2 + h(W20-W22) + u21              (all summed in PSUM)

Sharding: dest nodes are packed into T=1096 tiles of <=96 nodes, balanced
so every (tile, src-chunk) cell has <=~240 edge slots (2 blocks of 128).
Each of the 8 cores owns 137 tiles. Sources live in 4 chunks of 25024 rows
(int16 gather indices).

Descriptor pairing: gather tables hold PAIR rows [feat[a] | feat[b]] so a
single 256B/512B descriptor feeds two edges of the same (tile, chunk)
cell. A greedy matcher pairs ~74% of edges; paired slots fill block 0 of
each cell (2 selector builds + 2 matmuls), unpaired slots use only the
lo half (1 build + 1 matmul).

Device pipeline per pass: dma_gather pair rows -> DVE builds bf16
S[slot, destcol] = norm * (iota == ld) -> PE accumulates per-tile psum
(+ fused dense epilogue) -> Act engine writes tiles out.
"""
import numpy as np
import ml_dtypes
from contextlib import ExitStack

import concourse.bass as bass
import concourse.bacc as bacc
import concourse.mybir as mybir
import concourse.tile as tile
from concourse.bass_utils import run_bass_kernel_spmd

# problem constants
N = 100000
E = 1600000
F_IN = 128
F_HID = 64
F_OUT = 40

P = 128
D = 96                  # dest nodes per tile
NCORES = 8
TPC = 137               # tiles per core
T_TILES = TPC * NCORES  # 1096
NCHUNK = 4
CH = 25024              # source rows per chunk (int16 safe)
GRP = 4                 # tiles per psum group (acc banks + 2 psum2 banks)
PAIR_CAP = 128          # max paired slots per cell (keeps block0 pure)
RADJ = 3                # max adjacencies per source in the matcher
MAXROWS = 32700         # table row budget (int16)
SCRATCH = 16384         # swdge ring: 1024 descs per gather call

F32 = mybir.dt.float32
BF16 = mybir.dt.bfloat16
I16 = mybir.dt.int16
BF = ml_dtypes.bfloat16


# ---------------------------------------------------------------------------
# host-side graph preprocessing
# ---------------------------------------------------------------------------

def _pack_tiles(col, chunk):
    """Assign dest nodes to T_TILES tiles (<=D nodes each), balancing the
    per-(tile, chunk) edge counts. Greedy min-max over 4 chunk dims."""
    d = np.zeros((N, NCHUNK), np.int32)
    np.add.at(d, (col, chunk), 1)
    deg_tot = d.sum(1)
    order = np.argsort(-deg_tot, kind="stable")
    loads = np.zeros((T_TILES, NCHUNK), np.int32)
    counts = np.zeros(T_TILES, np.int32)
    assign = np.full(N, -1, np.int32)
    BIG = 1 << 20
    for v in order:
        cand = loads + d[v][None, :]
        m = cand.max(1).astype(np.int64)
        m[counts >= D] = BIG
        t = int(np.argmin(m))
        assign[v] = t
        loads[t] += d[v]
        counts[t] += 1
    return assign


def _match_core_chunk(s, cellid, ld, nrm, ncells):
    """Greedy pair matching for one (core, chunk).

    s: local source id per edge (0..CH-1); cellid: local tile per edge;
    ld/nrm: dest column and weight per edge.
    Emits explicit SLOTS: a paired slot carries two edges of one cell that
    share a table pair-row; a single slot carries one edge on the row's lo
    half. Returns dict with pairs[nrows,2] and per-slot arrays.
    """
    ne = len(s)
    o = np.argsort(cellid, kind="stable")
    s_s, eid_s = s[o], o
    bounds = np.searchsorted(cellid[o], np.arange(ncells + 1))
    adj_used = np.zeros(CH, np.int16)
    partners = {}            # src -> list of (partner, row_id, my_half)
    self_row = {}            # src -> row_id of (v,v) adjacency
    pairs = []
    sl_row, sl_cell, sl_pair = [], [], []
    sl_ldlo, sl_nrmlo, sl_ldhi, sl_nrmhi = [], [], [], []

    def emit_pair(rid, ci, e_lo, e_hi):
        sl_row.append(rid)
        sl_cell.append(ci)
        sl_pair.append(True)
        sl_ldlo.append(ld[e_lo])
        sl_nrmlo.append(nrm[e_lo])
        sl_ldhi.append(ld[e_hi])
        sl_nrmhi.append(nrm[e_hi])

    def emit_single(rid, ci, e):
        sl_row.append(rid)
        sl_cell.append(ci)
        sl_pair.append(False)
        sl_ldlo.append(ld[e])
        sl_nrmlo.append(nrm[e])
        sl_ldhi.append(-1.0)
        sl_nrmhi.append(0.0)

    unmatched_by_src = {}
    for ci in range(ncells):
        a0, b0 = bounds[ci], bounds[ci + 1]
        if a0 == b0:
            continue
        merged = 0
        open_e = {}
        for k in range(a0, b0):
            open_e.setdefault(int(s_s[k]), []).append(int(eid_s[k]))
        # 1. self pairs (two edges of same source in this cell share a row)
        for v, lst in open_e.items():
            while len(lst) >= 2 and merged < PAIR_CAP:
                rid = self_row.get(v, -1)
                if rid < 0:
                    if adj_used[v] <= RADJ - 2 and len(pairs) < MAXROWS:
                        adj_used[v] += 2
                        rid = len(pairs)
                        pairs.append((v, v))
                        self_row[v] = rid
                    else:
                        break
                emit_pair(rid, ci, lst.pop(), lst.pop())
                merged += 1
        # 2. reuse existing adjacencies
        for v in list(open_e.keys()):
            lst = open_e[v]
            if not lst or merged >= PAIR_CAP:
                continue
            for p, rid, half_v in partners.get(v, ()):
                if not lst or merged >= PAIR_CAP:
                    break
                plst = open_e.get(p)
                if p != v and plst:
                    ev, ep = lst.pop(), plst.pop()
                    if half_v == 0:
                        emit_pair(rid, ci, ev, ep)
                    else:
                        emit_pair(rid, ci, ep, ev)
                    merged += 1
        # 3. create new adjacencies among remaining
        flat = [(v, ei) for v, lst in open_e.items() for ei in lst]
        free, stuck = [], []
        for v, ei in flat:
            (free if adj_used[v] < RADJ and len(pairs) < MAXROWS
             else stuck).append((v, ei))
        while len(free) >= 2 and merged < PAIR_CAP and len(pairs) < MAXROWS:
            v1, e1 = free.pop()
            if free[-1][0] == v1:
                k = next((i for i in range(len(free)) if free[i][0] != v1), -1)
                if k < 0:
                    stuck.append((v1, e1))
                    stuck.extend(free)
                    free = []
                    break
                free[k], free[-1] = free[-1], free[k]
            v2, e2 = free.pop()
            adj_used[v1] += 1
            adj_used[v2] += 1
            rid = len(pairs)
            pairs.append((v1, v2))
            partners.setdefault(v1, []).append((v2, rid, 0))
            partners.setdefault(v2, []).append((v1, rid, 1))
            emit_pair(rid, ci, e1, e2)
            merged += 1
        stuck.extend(free)
        for v, ei in stuck:
            unmatched_by_src.setdefault(v, []).append((ci, ei))

    # unmatched edges: single slots on a (v,v) row's lo half
    for v, lst in unmatched_by_src.items():
        rid = self_row.get(v, -1)
        if rid < 0:
            rid = len(pairs)
            pairs.append((v, v))
            self_row[v] = rid
        for ci, ei in lst:
            emit_single(rid, ci, ei)
    assert len(pairs) <= 32767, len(pairs)
    nsl = len(sl_row)
    assert nsl == 0 or 2 * sum(sl_pair) + (nsl - sum(sl_pair)) == ne
    return dict(
        pairs=np.array(pairs, np.int64).reshape(-1, 2),
        srow=np.array(sl_row, np.int64),
        scell=np.array(sl_cell, np.int64),
        spair=np.array(sl_pair, bool),
        ldlo=np.array(sl_ldlo, np.float32),
        nrmlo=np.array(sl_nrmlo, np.float32),
        ldhi=np.array(sl_ldhi, np.float32),
        nrmhi=np.array(sl_nrmhi, np.float32),
    )


def _prep_graph(edge_index, edge_weight):
    row = np.ascontiguousarray(edge_index[0]).astype(np.int64)
    col = np.ascontiguousarray(edge_index[1]).astype(np.int64)
    w = np.ascontiguousarray(edge_weight).astype(np.float32)

    deg = np.bincount(row, weights=w.astype(np.float64), minlength=N).astype(np.float32)
    dinv = np.where(deg > 0, 1.0 / np.sqrt(np.maximum(deg, 1e-30)), 0.0).astype(np.float32)
    norm = (-dinv[row] * w * dinv[col]).astype(np.float32)

    chunk = np.minimum(row // CH, NCHUNK - 1)
    assign = _pack_tiles(col, chunk)

    # dest-local column within tile; node <-> (tile, col) maps
    order = np.argsort(assign, kind="stable")
    t_sorted = assign[order]
    start = np.searchsorted(t_sorted, np.arange(T_TILES + 1))
    ldcol = np.zeros(N, np.int64)
    tile_nodes = np.full((T_TILES, D), -1, np.int64)
    for t in range(T_TILES):
        nodes = order[start[t]:start[t + 1]]
        ldcol[nodes] = np.arange(len(nodes))
        tile_nodes[t, :len(nodes)] = nodes

    tile_of_edge = assign[col]
    core_of_edge = tile_of_edge // TPC
    ld_of_edge = ldcol[col]

    # per (core, chunk) matching
    cores = []
    NR = 0
    nslots_all = np.zeros((NCORES, TPC, NCHUNK), np.int64)
    for c in range(NCORES):
        csel = np.nonzero(core_of_edge == c)[0]
        per_chunk = []
        for ch in range(NCHUNK):
            sel = csel[chunk[csel] == ch]
            s_local = (row[sel] - ch * CH).astype(np.int64)
            cellid = (tile_of_edge[sel] - c * TPC).astype(np.int64)
            st = _match_core_chunk(s_local, cellid, ld_of_edge[sel],
                                   norm[sel], TPC)
            per_chunk.append(st)
            NR = max(NR, len(st["pairs"]))
            nslots_all[c, :, ch] = np.bincount(st["scell"], minlength=TPC)
        cores.append(per_chunk)
    nb_all = np.maximum(1, -(-nslots_all // P))

    # rank-align tiles across cores: sort each core's tiles by total nb
    tile_perm = np.zeros((NCORES, TPC), np.int64)
    for c in range(NCORES):
        tile_perm[c] = np.lexsort((np.arange(TPC), -nb_all[c].sum(1)))
    nb_sorted = np.stack([nb_all[c][tile_perm[c]] for c in range(NCORES)])
    NB = nb_sorted.max(0)                     # [TPC(pos), NCHUNK]
    B_TOTAL = int(NB.sum())
    SLOTS = B_TOTAL * P

    # shared block table + call plan (GRP positions, chunk-major runs).
    # hardware SWDGE ring holds 1024 descriptors -> gather calls <= 8 blocks.
    MAXBLK = SCRATCH // (16 * P)
    block_pos = []
    block_ch = []
    block_bi = []
    calls = []                                # (block0, nblocks, ch)
    groups = []                               # (pos0, npos)
    b = 0
    for g0 in range(0, TPC, GRP):
        gn = min(GRP, TPC - g0)
        groups.append((g0, gn))
        for ch in range(NCHUNK):
            nbk = int(NB[g0:g0 + gn, ch].sum())
            sub = []
            o = 0
            while o < nbk:
                n1 = min(MAXBLK, nbk - o)
                sub.append((b + o, n1))
                o += n1
            calls.append((sub, ch))
            for pos in range(g0, g0 + gn):
                for bi in range(int(NB[pos, ch])):
                    block_pos.append(pos)
                    block_ch.append(ch)
                    block_bi.append(bi)
                    b += 1
    assert b == B_TOTAL

    # per-core slot arrays
    blk_of = {}
    for bb in range(B_TOTAL):
        blk_of.setdefault((block_pos[bb], block_ch[bb]), []).append(bb)

    idx16 = np.zeros((NCORES, SLOTS), np.int16)
    meta = np.zeros((NCORES, P, 4 * B_TOTAL), np.float32)
    meta[:, :, 0::4] = -1.0                   # ld_lo
    meta[:, :, 2::4] = -1.0                   # ld_hi
    first_blk = {k: v[0] for k, v in blk_of.items()}
    for c in range(NCORES):
        inv_pos = np.zeros(TPC, np.int64)
        inv_pos[tile_perm[c]] = np.arange(TPC)
        for ch in range(NCHUNK):
            st = cores[c][ch]
            nslot = len(st["srow"])
            if nslot == 0:
                continue
            # order: by cell, paired slots first within cell
            ordr = np.lexsort((np.arange(nslot), ~st["spair"], st["scell"]))
            cell_s = st["scell"][ordr]
            pair_s = st["spair"][ordr]
            # global slot position for each ordered slot
            cb = np.searchsorted(cell_s, np.arange(TPC + 1))
            within = np.arange(nslot) - cb[cell_s]
            base_blk = np.array([first_blk[(int(inv_pos[tl]), ch)]
                                 for tl in range(TPC)], np.int64)
            npaired_cell = np.bincount(cell_s[pair_s], minlength=TPC)
            assert npaired_cell.max() <= P, npaired_cell.max()
            blk = base_blk[cell_s] + within // P
            prt = within % P
            slot = blk * P + prt
            idx16[c, slot] = st["srow"][ordr].astype(np.int16)
            meta[c, prt, 4 * blk] = st["ldlo"][ordr]
            meta[c, prt, 4 * blk + 1] = st["nrmlo"][ordr]
            meta[c, prt, 4 * blk + 2] = st["ldhi"][ordr]
            meta[c, prt, 4 * blk + 3] = st["nrmhi"][ordr]

    # wrapped idx layout [16, SLOTS/16] tiled to 128 partitions
    ii = np.arange(SLOTS)
    idxw = np.zeros((NCORES, 16, SLOTS // 16), np.int16)
    idxw[:, ii % 16, ii // 16] = idx16
    idxw = np.tile(idxw, (1, 8, 1))

    # node <-> (core, pos, ldcol) output mapping
    pos_of_tile = np.zeros(T_TILES, np.int64)
    for c in range(NCORES):
        pos_of_tile[tile_perm[c] + c * TPC] = np.arange(TPC)
    vnodes = np.arange(N)
    gi_core = assign[vnodes] // TPC
    gi_pos = pos_of_tile[assign[vnodes]]
    gi_j = ldcol[vnodes]
    # per-core pos-major node list [TPC, D] (entry: node id or N for pad)
    nodelist = np.full((NCORES, TPC, D), N, np.int64)
    for c in range(NCORES):
        tl = tile_perm[c] + c * TPC
        tn = tile_nodes[tl]                  # [TPC, D]
        nodelist[c] = np.where(tn >= 0, tn, N)

    return dict(cores=cores, NR=NR, NB=NB, B_TOTAL=B_TOTAL, SLOTS=SLOTS,
                calls=calls, groups=groups,
                block_pos=np.array(block_pos), block_ch=np.array(block_ch),
                block_bi=np.array(block_bi),
                idxw=idxw, meta=meta, idx16=idx16,
                gi_core=gi_core, gi_pos=gi_pos, gi_j=gi_j,
                nodelist=nodelist)


# ---------------------------------------------------------------------------
# device program builder
# ---------------------------------------------------------------------------

def _build_pass(g, mode, WROW, FP, HIOFF, Wo=None, KD=None, relu=False,
                out_f32=False):
    """One propagation pass.

    mode 'TP': psum acc [FP, D] (transposed); epilogue out = acc^T @ wcat
               (+ ones @ bcat) -> [D, Wo] tiles.
    mode 'CB': psum acc [D, FP]; epilogue acc += inT_tile^T @ wd
               + ident @ addin_tile; out = act(acc) -> [D, FP] tiles.
    """
    NR, NB, B_TOTAL, SLOTS = g["NR"], g["NB"], g["B_TOTAL"], g["SLOTS"]
    calls, groups = g["calls"], g["groups"]
    block_pos, block_ch, block_bi = g["block_pos"], g["block_ch"], g["block_bi"]

    nc = bacc.Bacc("TRN2", target_bir_lowering=False,
                   dynamic_dma_scratch_size=SCRATCH)
    tab = nc.declare_dram_parameter("tab", [NCHUNK * NR, WROW], BF16, isOutput=False)
    idx = nc.declare_dram_parameter("idx", [P, SLOTS // 16], I16, isOutput=False)
    meta = nc.declare_dram_parameter("meta", [P, 4 * B_TOTAL], F32, isOutput=False)
    iot = nc.declare_dram_parameter("iot", [P, D], BF16, isOutput=False)
    if mode == "TP":
        wcat = nc.declare_dram_parameter("wcat", [FP, Wo], BF16, isOutput=False)
        bcat = nc.declare_dram_parameter("bcat", [1, Wo], BF16, isOutput=False)
        out = nc.declare_dram_parameter("out", [D, TPC, Wo], BF16, isOutput=True)
        WO = Wo
    else:
        wd = nc.declare_dram_parameter("wd", [KD, FP], BF16, isOutput=False)
        inT = nc.declare_dram_parameter("inT", [KD, TPC * D], BF16, isOutput=False)
        addin = nc.declare_dram_parameter("addin", [D, TPC, FP], BF16, isOutput=False)
        ident = nc.declare_dram_parameter("ident", [D, D], BF16, isOutput=False)
        odt = F32 if out_f32 else BF16
        out = nc.declare_dram_parameter("out", [D, TPC, FP], odt, isOutput=True)
        WO = FP

    with ExitStack() as ctx:
        tc = ctx.enter_context(tile.TileContext(nc))
        cpool = ctx.enter_context(tc.tile_pool(name="const", bufs=1))
        gpool = ctx.enter_context(tc.tile_pool(name="g", bufs=6))
        spool = ctx.enter_context(tc.tile_pool(name="s", bufs=16))
        sbpool = ctx.enter_context(tc.tile_pool(name="sb", bufs=4))
        stpool = ctx.enter_context(tc.tile_pool(name="st", bufs=3))
        apool = ctx.enter_context(tc.tile_pool(name="acc", bufs=GRP, space="PSUM"))
        if mode == "TP":
            p2pool = ctx.enter_context(tc.tile_pool(name="p2", bufs=2, space="PSUM"))

        idx_t = cpool.tile([P, SLOTS // 16], I16)
        meta_t = cpool.tile([P, 4 * B_TOTAL], F32)
        IH = (SLOTS // 16) // 2
        MH = (4 * B_TOTAL) // 2
        nc.sync.dma_start(out=idx_t[:, :IH], in_=idx[:, :IH])
        nc.sync.dma_start(out=meta_t[:, :MH], in_=meta[:, :MH])
        nc.sync.dma_start(out=idx_t[:, IH:], in_=idx[:, IH:])
        nc.sync.dma_start(out=meta_t[:, MH:], in_=meta[:, MH:])
        iota_b = cpool.tile([P, D], BF16)
        nc.sync.dma_start(out=iota_b[:], in_=iot[:])
        if mode == "TP":
            wcat_t = cpool.tile([FP, Wo], BF16)
            bcat_t = cpool.tile([1, Wo], BF16)
            ones_t = cpool.tile([1, D], BF16)
            nc.sync.dma_start(out=wcat_t[:], in_=wcat[:])
            nc.sync.dma_start(out=bcat_t[:], in_=bcat[:])
            nc.vector.memset(ones_t[:], 1.0)
        else:
            wd_t = cpool.tile([KD, FP], BF16)
            ident_t = cpool.tile([D, D], BF16)
            nc.sync.dma_start(out=wd_t[:], in_=wd[:])
            nc.sync.dma_start(out=ident_t[:], in_=ident[:])
            inpool = ctx.enter_context(tc.tile_pool(name="inp", bufs=3))
            adpool = ctx.enter_context(tc.tile_pool(name="adp", bufs=3))

        acc = {}
        ci = 0
        for (g0, gn) in groups:
            if mode == "CB":
                int_g = inpool.tile([KD, GRP * D], BF16, tag="inp")
                ad_g = adpool.tile([D, GRP, FP], BF16, tag="adp")
                nc.sync.dma_start(out=int_g[:, :gn * D],
                                  in_=inT[:, g0 * D:(g0 + gn) * D])
                nc.sync.dma_start(out=ad_g[:, :gn, :],
                                  in_=addin[:, g0:g0 + gn, :])
            for ch in range(NCHUNK):
                sub, _ch = calls[ci]
                ci += 1
                for (b0, nbk) in sub:
                    gt = gpool.tile([P, nbk, WROW], BF16, tag="g")
                    nc.gpsimd.dma_gather(
                        gt[:], tab[ch * NR:(ch + 1) * NR, :],
                        idx_t[:, b0 * 8:(b0 + nbk) * 8],
                        nbk * P, nbk * P, WROW,
                    )
                    for j in range(nbk):
                        bb = b0 + j
                    pos = int(block_pos[bb])
                    bi = int(block_bi[bb])
                    first = (ch == 0 and bi == 0)
                    last_s = (ch == NCHUNK - 1 and bi == int(NB[pos, ch]) - 1)
                    if first:
                        shape = [FP, D] if mode == "TP" else [D, FP]
                        acc[pos] = apool.tile(shape, F32, space="PSUM",
                                              tag="acc", name="acc")
                    mcol = 4 * bb
                    S_lo = spool.tile([P, D], BF16, tag="S")
                    nc.vector.tensor_scalar(
                        out=S_lo[:], in0=iota_b[:],
                        scalar1=meta_t[:, mcol:mcol + 1],
                        scalar2=meta_t[:, mcol + 1:mcol + 2],
                        op0=mybir.AluOpType.is_equal,
                        op1=mybir.AluOpType.mult,
                    )
                    # hi selector only on block 0 (paired slots live there)
                    do_hi = (bi == 0)
                    if mode == "TP":
                        nc.tensor.matmul(out=acc[pos][:], lhsT=gt[:, j, 0:FP],
                                         rhs=S_lo[:], start=first,
                                         stop=(mode == "TP" and last_s and not do_hi))
                    else:
                        nc.tensor.matmul(out=acc[pos][:], lhsT=S_lo[:],
                                         rhs=gt[:, j, 0:FP], start=first,
                                         stop=False)
                    if do_hi:
                        S_hi = spool.tile([P, D], BF16, tag="S")
                        nc.vector.tensor_scalar(
                            out=S_hi[:], in0=iota_b[:],
                            scalar1=meta_t[:, mcol + 2:mcol + 3],
                            scalar2=meta_t[:, mcol + 3:mcol + 4],
                            op0=mybir.AluOpType.is_equal,
                            op1=mybir.AluOpType.mult,
                        )
                        if mode == "TP":
                            nc.tensor.matmul(out=acc[pos][:],
                                             lhsT=gt[:, j, HIOFF:HIOFF + FP],
                                             rhs=S_hi[:], start=False,
                                             stop=last_s)
                        else:
                            nc.tensor.matmul(out=acc[pos][:], lhsT=S_hi[:],
                                             rhs=gt[:, j, HIOFF:HIOFF + FP],
                                             start=False, stop=False)
            # group epilogue
            st = stpool.tile([D, GRP, WO], F32 if (mode == "CB" and out_f32) else BF16, tag="st")
            for k in range(gn):
                pos = g0 + k
                if mode == "TP":
                    sb = sbpool.tile([FP, D], BF16, tag="sb")
                    nc.scalar.activation(sb[:], acc[pos][:],
                                         mybir.ActivationFunctionType.Copy)
                    p2 = p2pool.tile([D, Wo], F32, space="PSUM", tag="p2")
                    nc.tensor.matmul(out=p2[:], lhsT=sb[:], rhs=wcat_t[:],
                                     start=True, stop=False)
                    nc.tensor.matmul(out=p2[:], lhsT=ones_t[:], rhs=bcat_t[:],
                                     start=False, stop=True)
                    nc.scalar.activation(st[:, k, :], p2[:],
                                         mybir.ActivationFunctionType.Copy)
                else:
                    nc.tensor.matmul(out=acc[pos][:],
                                     lhsT=int_g[:, k * D:(k + 1) * D],
                                     rhs=wd_t[:], start=False, stop=False)
                    nc.tensor.matmul(out=acc[pos][:], lhsT=ident_t[:],
                                     rhs=ad_g[:, k, :],
                                     start=False, stop=True)
                    fn = (mybir.ActivationFunctionType.Relu if relu
                          else mybir.ActivationFunctionType.Copy)
                    nc.scalar.activation(st[:, k, :], acc[pos][:], fn)
                del acc[pos]
            nc.sync.dma_start(out=out[:, g0:g0 + gn, :], in_=st[:, :gn, :])

    nc.compile()
    return nc


# ---------------------------------------------------------------------------
# host glue
# ---------------------------------------------------------------------------

def _to_bf(a):
    return np.asarray(a, np.float32).astype(BF)


def _build_tables(g, feat, wrow, fp, hioff):
    """Per-core gather tables [NCHUNK*NR, wrow] from full features
    feat [N, fw] (fw = feat width <= fp slots in the row)."""
    NR = g["NR"]
    fw = feat.shape[1]
    featp = np.zeros((NCHUNK * CH, fw), BF)
    featp[:N] = feat
    tabs = []
    for c in range(NCORES):
        t = np.zeros((NCHUNK * NR, wrow), BF)
        for ch in range(NCHUNK):
            pairs = g["cores"][c][ch]["pairs"]
            nr = len(pairs)
            if nr == 0:
                continue
            base = featp[ch * CH:(ch + 1) * CH]
            t[ch * NR:ch * NR + nr, 0:fw] = base[pairs[:, 0]]
            t[ch * NR:ch * NR + nr, hioff:hioff + fw] = base[pairs[:, 1]]
        tabs.append(t)
    return tabs


def _unpermute(g, outs, fw):
    """outs: list of 8 arrays [D, TPC, >=fw] -> full [N, fw] float32."""
    stack = np.stack([np.asarray(o)[:, :, :fw] for o in outs])  # [C, D, TPC, fw]
    return stack[g["gi_core"], g["gi_j"], g["gi_pos"], :].astype(np.float32)


def _run(nc, in_maps):
    res = run_bass_kernel_spmd(nc, in_maps, list(range(NCORES)))
    return res.results


class _Programs:
    def __init__(self, g):
        self.g = g
        self.p1 = _build_pass(g, "TP", WROW=2 * F_IN, FP=F_IN, HIOFF=F_IN,
                              Wo=2 * F_HID)
        self.p2 = _build_pass(g, "CB", WROW=2 * F_HID, FP=F_HID, HIOFF=F_HID,
                              KD=F_IN, relu=True)
        self.p3 = _build_pass(g, "TP", WROW=2 * F_HID, FP=F_HID, HIOFF=F_HID,
                              Wo=2 * F_OUT)
        self.p4 = _build_pass(g, "CB", WROW=2 * F_HID, FP=F_OUT, HIOFF=F_OUT,
                              KD=F_HID, relu=False, out_f32=True)


def kernel(x, edge_index, edge_weight, W1, b1, W2, b2):
    x = np.asarray(x, np.float32)
    edge_index = np.asarray(edge_index)
    edge_weight = np.asarray(edge_weight, np.float32)
    W1 = np.asarray(W1, np.float32)
    b1 = np.asarray(b1, np.float32)
    W2 = np.asarray(W2, np.float32)
    b2 = np.asarray(b2, np.float32)

    g = _prep_graph(edge_index, edge_weight)
    progs = _Programs(g)
    return _run_all(g, progs, x, W1, b1, W2, b2)


_IOTA = np.tile(np.arange(D, dtype=np.float32).astype(BF)[None, :], (P, 1))


def _core_inputs(g, c, tabs, extra):
    m = {"tab": tabs[c], "idx": g["idxw"][c], "meta": g["meta"][c],
         "iot": _IOTA}
    m.update(extra(c) if callable(extra) else extra)
    return m


def _run_all(g, progs, x, W1, b1, W2, b2):
    xb = _to_bf(x)
    nodelist = g["nodelist"]                    # [C, TPC, D] node or N
    xpad = np.zeros((N + 1, F_IN), BF)
    xpad[:N] = xb

    # P1: propagate x; per tile emit [y1 | t11] = agg^T [2W12 | W11] + [0|b1]
    w1cat = np.concatenate([2.0 * W1[2], W1[1]], axis=1)        # [128, 128]
    b1cat = np.concatenate([np.zeros(F_HID, np.float32), b1])[None, :]
    tabs = _build_tables(g, xb, 2 * F_IN, F_IN, F_IN)
    maps = [_core_inputs(g, c, tabs,
                         {"wcat": w1cat.astype(BF), "bcat": b1cat.astype(BF)})
            for c in range(NCORES)]
    res = _run(progs.p1, maps)
    out1 = [np.asarray(r["out"]) for r in res]          # [D, TPC, 128] bf16
    y1_full = _unpermute(g, out1, F_HID).astype(BF)     # 2(Lx)W12 rows

    # P2: h = relu(L y1 + x (W10-W12) + t11)
    w2d = (W1[0] - W1[2]).astype(BF)                     # [128, 64]
    idn = np.eye(D, dtype=np.float32).astype(BF)
    tabs = _build_tables(g, y1_full, 2 * F_HID, F_HID, F_HID)

    def p2_extra(c):
        xt = xpad[nodelist[c].reshape(-1)]               # [TPC*D, 128]
        return {"wd": w2d,
                "inT": np.ascontiguousarray(xt.T).astype(BF),
                "addin": np.ascontiguousarray(
                    out1[c][:, :, F_HID:]).astype(BF),
                "ident": idn}
    maps = [_core_inputs(g, c, tabs, p2_extra) for c in range(NCORES)]
    res = _run(progs.p2, maps)
    outh = [np.asarray(r["out"]) for r in res]           # [D, TPC, 64] bf16
    h_full = _unpermute(g, outh, F_HID).astype(BF)

    # P3: propagate h; per tile emit [y2 | u21b]
    w2cat = np.concatenate([2.0 * W2[2], W2[1]], axis=1)         # [64, 80]
    b2cat = np.concatenate([np.zeros(F_OUT, np.float32), b2])[None, :]
    tabs = _build_tables(g, h_full, 2 * F_HID, F_HID, F_HID)
    maps = [_core_inputs(g, c, tabs,
                         {"wcat": w2cat.astype(BF), "bcat": b2cat.astype(BF)})
            for c in range(NCORES)]
    res = _run(progs.p3, maps)
    out3 = [np.asarray(r["out"]) for r in res]           # [D, TPC, 80] bf16
    y2_full = _unpermute(g, out3, F_OUT).astype(BF)

    # P4: out = L y2 + h (W20-W22) + u21b
    w4d = (W2[0] - W2[2]).astype(BF)                     # [64, 40]
    hpad = np.zeros((N + 1, F_HID), BF)
    hpad[:N] = h_full
    tabs = _build_tables(g, y2_full, 2 * F_HID, F_OUT, F_OUT)

    def p4_extra(c):
        ht = hpad[nodelist[c].reshape(-1)]
        return {"wd": w4d,
                "inT": np.ascontiguousarray(ht.T).astype(BF),
                "addin": np.ascontiguousarray(
                    out3[c][:, :, F_OUT:]).astype(BF),
                "ident": idn}
    maps = [_core_inputs(g, c, tabs, p4_extra) for c in range(NCORES)]
    res = _run(progs.p4, maps)
    out4 = [np.asarray(r["out"]) for r in res]           # [D, TPC, 40] f32
    return _unpermute(g, out4, F_OUT)


# revision 3
# speedup vs baseline: 1.1143x; 1.0042x over previous
"""ChebNet (2-layer ChebConv, K=3) on 8 Trainium2 NeuronCores — v3.

Math: propagation commutes with feature matmuls, so per layer
    out = x(W0-W2) + (Lx)W1 + 2 L((Lx)W2) + b
giving 4 sparse propagations total (2 per layer) plus small dense matmuls
that are fused into the PSUM accumulation of each propagation pass:
  P1: agg1 = L x           -> per tile: [y1 | t11] = agg1^T [2W12 | W11] (+b1)
  P2: h = relu(L y1 + x(W10-W12) + t11)          (all summed in PSUM)
  P3: agg3 = L h           -> per tile: [y2 | u21] = agg3^T [2W22 | W21] (+b2)
  P4: out = L y2 + h(W20-W22) + u21              (all summed in PSUM)

Sharding: dest nodes are packed into T=1096 tiles of <=96 nodes, balanced
so every (tile, src-chunk) cell has <=~240 edge slots (2 blocks of 128).
Each of the 8 cores owns 137 tiles. Sources live in 4 chunks of 25024 rows
(int16 gather indices).

Descriptor pairing: gather tables hold PAIR rows [feat[a] | feat[b]] so a
single 256B/512B descriptor feeds two edges of the same (tile, chunk)
cell. A greedy matcher pairs ~74% of edges; paired slots fill block 0 of
each cell (2 selector builds + 2 matmuls), unpaired slots use only the
lo half (1 build + 1 matmul).

Device pipeline per pass: dma_gather pair rows -> DVE builds bf16
S[slot, destcol] = norm * (iota == ld) -> PE accumulates per-tile psum
(+ fused dense epilogue) -> Act engine writes tiles out.
"""
import numpy as np
import ml_dtypes
from contextlib import ExitStack

import concourse.bass as bass
import concourse.bacc as bacc
import concourse.mybir as mybir
import concourse.tile as tile
from concourse.bass_utils import run_bass_kernel_spmd

# problem constants
N = 100000
E = 1600000
F_IN = 128
F_HID = 64
F_OUT = 40

P = 128
D = 96                  # dest nodes per tile
NCORES = 8
TPC = 137               # tiles per core
T_TILES = TPC * NCORES  # 1096
NCHUNK = 4
CH = 25024              # source rows per chunk (int16 safe)
GRP = 4                 # tiles per psum group (acc banks + 2 psum2 banks)
PAIR_CAP = 128          # max paired slots per cell (keeps block0 pure)
RADJ = 3                # max adjacencies per source in the matcher
MAXROWS = 32700         # table row budget (int16)
SCRATCH = 16384         # swdge ring: 1024 descs per gather call

F32 = mybir.dt.float32
BF16 = mybir.dt.bfloat16
I16 = mybir.dt.int16
BF = ml_dtypes.bfloat16


# ---------------------------------------------------------------------------
# host-side graph preprocessing
# ---------------------------------------------------------------------------

def _pack_tiles(col, chunk):
    """Assign dest nodes to T_TILES tiles (<=D nodes each), balancing the
    per-(tile, chunk) edge counts. Greedy min-max over 4 chunk dims."""
    d = np.zeros((N, NCHUNK), np.int32)
    np.add.at(d, (col, chunk), 1)
    deg_tot = d.sum(1)
    order = np.argsort(-deg_tot, kind="stable")
    loads = np.zeros((T_TILES, NCHUNK), np.int32)
    counts = np.zeros(T_TILES, np.int32)
    assign = np.full(N, -1, np.int32)
    BIG = 1 << 20
    for v in order:
        cand = loads + d[v][None, :]
        m = cand.max(1).astype(np.int64)
        m[counts >= D] = BIG
        t = int(np.argmin(m))
        assign[v] = t
        loads[t] += d[v]
        counts[t] += 1
    return assign


def _match_core_chunk(s, cellid, ld, nrm, ncells):
    """Greedy pair matching for one (core, chunk).

    s: local source id per edge (0..CH-1); cellid: local tile per edge;
    ld/nrm: dest column and weight per edge.
    Emits explicit SLOTS: a paired slot carries two edges of one cell that
    share a table pair-row; a single slot carries one edge on the row's lo
    half. Returns dict with pairs[nrows,2] and per-slot arrays.
    """
    ne = len(s)
    o = np.argsort(cellid, kind="stable")
    s_s, eid_s = s[o], o
    bounds = np.searchsorted(cellid[o], np.arange(ncells + 1))
    adj_used = np.zeros(CH, np.int16)
    partners = {}            # src -> list of (partner, row_id, my_half)
    self_row = {}            # src -> row_id of (v,v) adjacency
    pairs = []
    sl_row, sl_cell, sl_pair = [], [], []
    sl_ldlo, sl_nrmlo, sl_ldhi, sl_nrmhi = [], [], [], []

    def emit_pair(rid, ci, e_lo, e_hi):
        sl_row.append(rid)
        sl_cell.append(ci)
        sl_pair.append(True)
        sl_ldlo.append(ld[e_lo])
        sl_nrmlo.append(nrm[e_lo])
        sl_ldhi.append(ld[e_hi])
        sl_nrmhi.append(nrm[e_hi])

    def emit_single(rid, ci, e):
        sl_row.append(rid)
        sl_cell.append(ci)
        sl_pair.append(False)
        sl_ldlo.append(ld[e])
        sl_nrmlo.append(nrm[e])
        sl_ldhi.append(-1.0)
        sl_nrmhi.append(0.0)

    unmatched_by_src = {}
    for ci in range(ncells):
        a0, b0 = bounds[ci], bounds[ci + 1]
        if a0 == b0:
            continue
        merged = 0
        open_e = {}
        for k in range(a0, b0):
            open_e.setdefault(int(s_s[k]), []).append(int(eid_s[k]))
        # 1. self pairs (two edges of same source in this cell share a row)
        for v, lst in open_e.items():
            while len(lst) >= 2 and merged < PAIR_CAP:
                rid = self_row.get(v, -1)
                if rid < 0:
                    if adj_used[v] <= RADJ - 2 and len(pairs) < MAXROWS:
                        adj_used[v] += 2
                        rid = len(pairs)
                        pairs.append((v, v))
                        self_row[v] = rid
                    else:
                        break
                emit_pair(rid, ci, lst.pop(), lst.pop())
                merged += 1
        # 2. reuse existing adjacencies
        for v in list(open_e.keys()):
            lst = open_e[v]
            if not lst or merged >= PAIR_CAP:
                continue
            for p, rid, half_v in partners.get(v, ()):
                if not lst or merged >= PAIR_CAP:
                    break
                plst = open_e.get(p)
                if p != v and plst:
                    ev, ep = lst.pop(), plst.pop()
                    if half_v == 0:
                        emit_pair(rid, ci, ev, ep)
                    else:
                        emit_pair(rid, ci, ep, ev)
                    merged += 1
        # 3. create new adjacencies among remaining
        flat = [(v, ei) for v, lst in open_e.items() for ei in lst]
        free, stuck = [], []
        for v, ei in flat:
            (free if adj_used[v] < RADJ and len(pairs) < MAXROWS
             else stuck).append((v, ei))
        while len(free) >= 2 and merged < PAIR_CAP and len(pairs) < MAXROWS:
            v1, e1 = free.pop()
            if free[-1][0] == v1:
                k = next((i for i in range(len(free)) if free[i][0] != v1), -1)
                if k < 0:
                    stuck.append((v1, e1))
                    stuck.extend(free)
                    free = []
                    break
                free[k], free[-1] = free[-1], free[k]
            v2, e2 = free.pop()
            adj_used[v1] += 1
            adj_used[v2] += 1
            rid = len(pairs)
            pairs.append((v1, v2))
            partners.setdefault(v1, []).append((v2, rid, 0))
            partners.setdefault(v2, []).append((v1, rid, 1))
            emit_pair(rid, ci, e1, e2)
            merged += 1
        stuck.extend(free)
        for v, ei in stuck:
            unmatched_by_src.setdefault(v, []).append((ci, ei))

    # unmatched edges: single slots on a (v,v) row's lo half
    for v, lst in unmatched_by_src.items():
        rid = self_row.get(v, -1)
        if rid < 0:
            rid = len(pairs)
            pairs.append((v, v))
            self_row[v] = rid
        for ci, ei in lst:
            emit_single(rid, ci, ei)
    assert len(pairs) <= 32767, len(pairs)
    nsl = len(sl_row)
    assert nsl == 0 or 2 * sum(sl_pair) + (nsl - sum(sl_pair)) == ne
    return dict(
        pairs=np.array(pairs, np.int64).reshape(-1, 2),
        srow=np.array(sl_row, np.int64),
        scell=np.array(sl_cell, np.int64),
        spair=np.array(sl_pair, bool),
        ldlo=np.array(sl_ldlo, np.float32),
        nrmlo=np.array(sl_nrmlo, np.float32),
        ldhi=np.array(sl_ldhi, np.float32),
        nrmhi=np.array(sl_nrmhi, np.float32),
    )


def _prep_graph(edge_index, edge_weight):
    row = np.ascontiguousarray(edge_index[0]).astype(np.int64)
    col = np.ascontiguousarray(edge_index[1]).astype(np.int64)
    w = np.ascontiguousarray(edge_weight).astype(np.float32)

    deg = np.bincount(row, weights=w.astype(np.float64), minlength=N).astype(np.float32)
    dinv = np.where(deg > 0, 1.0 / np.sqrt(np.maximum(deg, 1e-30)), 0.0).astype(np.float32)
    norm = (-dinv[row] * w * dinv[col]).astype(np.float32)

    chunk = np.minimum(row // CH, NCHUNK - 1)
    assign = _pack_tiles(col, chunk)

    # dest-local column within tile; node <-> (tile, col) maps
    order = np.argsort(assign, kind="stable")
    t_sorted = assign[order]
    start = np.searchsorted(t_sorted, np.arange(T_TILES + 1))
    ldcol = np.zeros(N, np.int64)
    tile_nodes = np.full((T_TILES, D), -1, np.int64)
    for t in range(T_TILES):
        nodes = order[start[t]:start[t + 1]]
        ldcol[nodes] = np.arange(len(nodes))
        tile_nodes[t, :len(nodes)] = nodes

    tile_of_edge = assign[col]
    core_of_edge = tile_of_edge // TPC
    ld_of_edge = ldcol[col]

    # per (core, chunk) matching
    cores = []
    NR = 0
    nslots_all = np.zeros((NCORES, TPC, NCHUNK), np.int64)
    for c in range(NCORES):
        csel = np.nonzero(core_of_edge == c)[0]
        per_chunk = []
        for ch in range(NCHUNK):
            sel = csel[chunk[csel] == ch]
            s_local = (row[sel] - ch * CH).astype(np.int64)
            cellid = (tile_of_edge[sel] - c * TPC).astype(np.int64)
            st = _match_core_chunk(s_local, cellid, ld_of_edge[sel],
                                   norm[sel], TPC)
            per_chunk.append(st)
            NR = max(NR, len(st["pairs"]))
            nslots_all[c, :, ch] = np.bincount(st["scell"], minlength=TPC)
        cores.append(per_chunk)
    nb_all = np.maximum(1, -(-nslots_all // P))

    # rank-align tiles across cores: sort each core's tiles by total nb
    tile_perm = np.zeros((NCORES, TPC), np.int64)
    for c in range(NCORES):
        tile_perm[c] = np.lexsort((np.arange(TPC), -nb_all[c].sum(1)))
    nb_sorted = np.stack([nb_all[c][tile_perm[c]] for c in range(NCORES)])
    NB = nb_sorted.max(0)                     # [TPC(pos), NCHUNK]
    B_TOTAL = int(NB.sum())
    SLOTS = B_TOTAL * P

    # shared block table + call plan (GRP positions, chunk-major runs).
    # hardware SWDGE ring holds 1024 descriptors -> gather calls <= 8 blocks.
    MAXBLK = SCRATCH // (16 * P)
    block_pos = []
    block_ch = []
    block_bi = []
    calls = []                                # (block0, nblocks, ch)
    groups = []                               # (pos0, npos)
    b = 0
    for g0 in range(0, TPC, GRP):
        gn = min(GRP, TPC - g0)
        groups.append((g0, gn))
        for ch in range(NCHUNK):
            nbk = int(NB[g0:g0 + gn, ch].sum())
            sub = []
            o = 0
            while o < nbk:
                n1 = min(MAXBLK, nbk - o)
                sub.append((b + o, n1))
                o += n1
            calls.append((sub, ch))
            for pos in range(g0, g0 + gn):
                for bi in range(int(NB[pos, ch])):
                    block_pos.append(pos)
                    block_ch.append(ch)
                    block_bi.append(bi)
                    b += 1
    assert b == B_TOTAL

    # per-core slot arrays
    blk_of = {}
    for bb in range(B_TOTAL):
        blk_of.setdefault((block_pos[bb], block_ch[bb]), []).append(bb)

    idx16 = np.zeros((NCORES, SLOTS), np.int16)
    meta = np.zeros((NCORES, P, 4 * B_TOTAL), np.float32)
    meta[:, :, 0::4] = -1.0                   # ld_lo
    meta[:, :, 2::4] = -1.0                   # ld_hi
    first_blk = {k: v[0] for k, v in blk_of.items()}
    for c in range(NCORES):
        inv_pos = np.zeros(TPC, np.int64)
        inv_pos[tile_perm[c]] = np.arange(TPC)
        for ch in range(NCHUNK):
            st = cores[c][ch]
            nslot = len(st["srow"])
            if nslot == 0:
                continue
            # order: by cell, paired slots first within cell
            ordr = np.lexsort((np.arange(nslot), ~st["spair"], st["scell"]))
            cell_s = st["scell"][ordr]
            pair_s = st["spair"][ordr]
            # global slot position for each ordered slot
            cb = np.searchsorted(cell_s, np.arange(TPC + 1))
            within = np.arange(nslot) - cb[cell_s]
            base_blk = np.array([first_blk[(int(inv_pos[tl]), ch)]
                                 for tl in range(TPC)], np.int64)
            npaired_cell = np.bincount(cell_s[pair_s], minlength=TPC)
            assert npaired_cell.max() <= P, npaired_cell.max()
            blk = base_blk[cell_s] + within // P
            prt = within % P
            slot = blk * P + prt
            idx16[c, slot] = st["srow"][ordr].astype(np.int16)
            meta[c, prt, 4 * blk] = st["ldlo"][ordr]
            meta[c, prt, 4 * blk + 1] = st["nrmlo"][ordr]
            meta[c, prt, 4 * blk + 2] = st["ldhi"][ordr]
            meta[c, prt, 4 * blk + 3] = st["nrmhi"][ordr]

    # wrapped idx layout [16, SLOTS/16] tiled to 128 partitions
    ii = np.arange(SLOTS)
    idxw = np.zeros((NCORES, 16, SLOTS // 16), np.int16)
    idxw[:, ii % 16, ii // 16] = idx16
    idxw = np.tile(idxw, (1, 8, 1))

    # node <-> (core, pos, ldcol) output mapping
    pos_of_tile = np.zeros(T_TILES, np.int64)
    for c in range(NCORES):
        pos_of_tile[tile_perm[c] + c * TPC] = np.arange(TPC)
    vnodes = np.arange(N)
    gi_core = assign[vnodes] // TPC
    gi_pos = pos_of_tile[assign[vnodes]]
    gi_j = ldcol[vnodes]
    # per-core pos-major node list [TPC, D] (entry: node id or N for pad)
    nodelist = np.full((NCORES, TPC, D), N, np.int64)
    for c in range(NCORES):
        tl = tile_perm[c] + c * TPC
        tn = tile_nodes[tl]                  # [TPC, D]
        nodelist[c] = np.where(tn >= 0, tn, N)

    return dict(cores=cores, NR=NR, NB=NB, B_TOTAL=B_TOTAL, SLOTS=SLOTS,
                calls=calls, groups=groups,
                block_pos=np.array(block_pos), block_ch=np.array(block_ch),
                block_bi=np.array(block_bi),
                idxw=idxw, meta=meta, idx16=idx16,
                gi_core=gi_core, gi_pos=gi_pos, gi_j=gi_j,
                nodelist=nodelist)


# ---------------------------------------------------------------------------
# device program builder
# ---------------------------------------------------------------------------

def _build_pass(g, mode, WROW, FP, HIOFF, Wo=None, KD=None, relu=False,
                out_f32=False):
    """One propagation pass.

    mode 'TP': psum acc [FP, D] (transposed); epilogue out = acc^T @ wcat
               (+ ones @ bcat) -> [D, Wo] tiles.
    mode 'CB': psum acc [D, FP]; epilogue acc += inT_tile^T @ wd
               + ident @ addin_tile; out = act(acc) -> [D, FP] tiles.
    """
    NR, NB, B_TOTAL, SLOTS = g["NR"], g["NB"], g["B_TOTAL"], g["SLOTS"]
    calls, groups = g["calls"], g["groups"]
    block_pos, block_ch, block_bi = g["block_pos"], g["block_ch"], g["block_bi"]

    nc = bacc.Bacc("TRN2", target_bir_lowering=False,
                   dynamic_dma_scratch_size=SCRATCH)
    tab = nc.declare_dram_parameter("tab", [NCHUNK * NR, WROW], BF16, isOutput=False)
    idx = nc.declare_dram_parameter("idx", [P, SLOTS // 16], I16, isOutput=False)
    meta = nc.declare_dram_parameter("meta", [P, 4 * B_TOTAL], F32, isOutput=False)
    iot = nc.declare_dram_parameter("iot", [P, D], BF16, isOutput=False)
    if mode == "TP":
        wcat = nc.declare_dram_parameter("wcat", [FP, Wo], BF16, isOutput=False)
        bcat = nc.declare_dram_parameter("bcat", [1, Wo], BF16, isOutput=False)
        out = nc.declare_dram_parameter("out", [D, TPC, Wo], BF16, isOutput=True)
        WO = Wo
    else:
        wd = nc.declare_dram_parameter("wd", [KD, FP], BF16, isOutput=False)
        inT = nc.declare_dram_parameter("inT", [KD, TPC * D], BF16, isOutput=False)
        addin = nc.declare_dram_parameter("addin", [D, TPC, FP], BF16, isOutput=False)
        ident = nc.declare_dram_parameter("ident", [D, D], BF16, isOutput=False)
        odt = F32 if out_f32 else BF16
        out = nc.declare_dram_parameter("out", [D, TPC, FP], odt, isOutput=True)
        WO = FP

    with ExitStack() as ctx:
        tc = ctx.enter_context(tile.TileContext(nc))
        cpool = ctx.enter_context(tc.tile_pool(name="const", bufs=1))
        gpool = ctx.enter_context(tc.tile_pool(name="g", bufs=8))
        spool = ctx.enter_context(tc.tile_pool(name="s", bufs=24))
        sbpool = ctx.enter_context(tc.tile_pool(name="sb", bufs=4))
        stpool = ctx.enter_context(tc.tile_pool(name="st", bufs=3))
        apool = ctx.enter_context(tc.tile_pool(name="acc", bufs=GRP, space="PSUM"))
        if mode == "TP":
            p2pool = ctx.enter_context(tc.tile_pool(name="p2", bufs=2, space="PSUM"))

        idx_t = cpool.tile([P, SLOTS // 16], I16)
        meta_t = cpool.tile([P, 4 * B_TOTAL], F32)
        IH = (SLOTS // 16) // 2
        MH = (4 * B_TOTAL) // 2
        nc.sync.dma_start(out=idx_t[:, :IH], in_=idx[:, :IH])
        nc.sync.dma_start(out=meta_t[:, :MH], in_=meta[:, :MH])
        nc.sync.dma_start(out=idx_t[:, IH:], in_=idx[:, IH:])
        nc.sync.dma_start(out=meta_t[:, MH:], in_=meta[:, MH:])
        iota_b = cpool.tile([P, D], BF16)
        nc.sync.dma_start(out=iota_b[:], in_=iot[:])
        if mode == "TP":
            wcat_t = cpool.tile([FP, Wo], BF16)
            bcat_t = cpool.tile([1, Wo], BF16)
            ones_t = cpool.tile([1, D], BF16)
            nc.sync.dma_start(out=wcat_t[:], in_=wcat[:])
            nc.sync.dma_start(out=bcat_t[:], in_=bcat[:])
            nc.vector.memset(ones_t[:], 1.0)
        else:
            wd_t = cpool.tile([KD, FP], BF16)
            ident_t = cpool.tile([D, D], BF16)
            nc.sync.dma_start(out=wd_t[:], in_=wd[:])
            nc.sync.dma_start(out=ident_t[:], in_=ident[:])
            inpool = ctx.enter_context(tc.tile_pool(name="inp", bufs=3))
            adpool = ctx.enter_context(tc.tile_pool(name="adp", bufs=3))

        acc = {}
        ci = 0
        for (g0, gn) in groups:
            if mode == "CB":
                int_g = inpool.tile([KD, GRP * D], BF16, tag="inp")
                ad_g = adpool.tile([D, GRP, FP], BF16, tag="adp")
                nc.sync.dma_start(out=int_g[:, :gn * D],
                                  in_=inT[:, g0 * D:(g0 + gn) * D])
                nc.sync.dma_start(out=ad_g[:, :gn, :],
                                  in_=addin[:, g0:g0 + gn, :])
            for ch in range(NCHUNK):
                sub, _ch = calls[ci]
                ci += 1
                for (b0, nbk) in sub:
                    gt = gpool.tile([P, nbk, WROW], BF16, tag="g")
                    nc.gpsimd.dma_gather(
                        gt[:], tab[ch * NR:(ch + 1) * NR, :],
                        idx_t[:, b0 * 8:(b0 + nbk) * 8],
                        nbk * P, nbk * P, WROW,
                    )
                    for j in range(nbk):
                        bb = b0 + j
                    pos = int(block_pos[bb])
                    bi = int(block_bi[bb])
                    first = (ch == 0 and bi == 0)
                    last_s = (ch == NCHUNK - 1 and bi == int(NB[pos, ch]) - 1)
                    if first:
                        shape = [FP, D] if mode == "TP" else [D, FP]
                        acc[pos] = apool.tile(shape, F32, space="PSUM",
                                              tag="acc", name="acc")
                    mcol = 4 * bb
                    S_lo = spool.tile([P, D], BF16, tag="S")
                    nc.vector.tensor_scalar(
                        out=S_lo[:], in0=iota_b[:],
                        scalar1=meta_t[:, mcol:mcol + 1],
                        scalar2=meta_t[:, mcol + 1:mcol + 2],
                        op0=mybir.AluOpType.is_equal,
                        op1=mybir.AluOpType.mult,
                    )
                    # hi selector only on block 0 (paired slots live there)
                    do_hi = (bi == 0)
                    if mode == "TP":
                        nc.tensor.matmul(out=acc[pos][:], lhsT=gt[:, j, 0:FP],
                                         rhs=S_lo[:], start=first,
                                         stop=(mode == "TP" and last_s and not do_hi))
                    else:
                        nc.tensor.matmul(out=acc[pos][:], lhsT=S_lo[:],
                                         rhs=gt[:, j, 0:FP], start=first,
                                         stop=False)
                    if do_hi:
                        S_hi = spool.tile([P, D], BF16, tag="S")
                        nc.vector.tensor_scalar(
                            out=S_hi[:], in0=iota_b[:],
                            scalar1=meta_t[:, mcol + 2:mcol + 3],
                            scalar2=meta_t[:, mcol + 3:mcol + 4],
                            op0=mybir.AluOpType.is_equal,
                            op1=mybir.AluOpType.mult,
                        )
                        if mode == "TP":
                            nc.tensor.matmul(out=acc[pos][:],
                                             lhsT=gt[:, j, HIOFF:HIOFF + FP],
                                             rhs=S_hi[:], start=False,
                                             stop=last_s)
                        else:
                            nc.tensor.matmul(out=acc[pos][:], lhsT=S_hi[:],
                                             rhs=gt[:, j, HIOFF:HIOFF + FP],
                                             start=False, stop=False)
            # group epilogue
            st = stpool.tile([D, GRP, WO], F32 if (mode == "CB" and out_f32) else BF16, tag="st")
            for k in range(gn):
                pos = g0 + k
                if mode == "TP":
                    sb = sbpool.tile([FP, D], BF16, tag="sb")
                    nc.scalar.activation(sb[:], acc[pos][:],
                                         mybir.ActivationFunctionType.Copy)
                    p2 = p2pool.tile([D, Wo], F32, space="PSUM", tag="p2")
                    nc.tensor.matmul(out=p2[:], lhsT=sb[:], rhs=wcat_t[:],
                                     start=True, stop=False)
                    nc.tensor.matmul(out=p2[:], lhsT=ones_t[:], rhs=bcat_t[:],
                                     start=False, stop=True)
                    nc.scalar.activation(st[:, k, :], p2[:],
                                         mybir.ActivationFunctionType.Copy)
                else:
                    nc.tensor.matmul(out=acc[pos][:],
                                     lhsT=int_g[:, k * D:(k + 1) * D],
                                     rhs=wd_t[:], start=False, stop=False)
                    nc.tensor.matmul(out=acc[pos][:], lhsT=ident_t[:],
                                     rhs=ad_g[:, k, :],
                                     start=False, stop=True)
                    fn = (mybir.ActivationFunctionType.Relu if relu
                          else mybir.ActivationFunctionType.Copy)
                    nc.scalar.activation(st[:, k, :], acc[pos][:], fn)
                del acc[pos]
            nc.sync.dma_start(out=out[:, g0:g0 + gn, :], in_=st[:, :gn, :])

    nc.compile()
    return nc


# ---------------------------------------------------------------------------
# host glue
# ---------------------------------------------------------------------------

def _to_bf(a):
    return np.asarray(a, np.float32).astype(BF)


def _build_tables(g, feat, wrow, fp, hioff):
    """Per-core gather tables [NCHUNK*NR, wrow] from full features
    feat [N, fw] (fw = feat width <= fp slots in the row)."""
    NR = g["NR"]
    fw = feat.shape[1]
    featp = np.zeros((NCHUNK * CH, fw), BF)
    featp[:N] = feat
    tabs = []
    for c in range(NCORES):
        t = np.zeros((NCHUNK * NR, wrow), BF)
        for ch in range(NCHUNK):
            pairs = g["cores"][c][ch]["pairs"]
            nr = len(pairs)
            if nr == 0:
                continue
            base = featp[ch * CH:(ch + 1) * CH]
            t[ch * NR:ch * NR + nr, 0:fw] = base[pairs[:, 0]]
            t[ch * NR:ch * NR + nr, hioff:hioff + fw] = base[pairs[:, 1]]
        tabs.append(t)
    return tabs


def _unpermute(g, outs, fw):
    """outs: list of 8 arrays [D, TPC, >=fw] -> full [N, fw] float32."""
    stack = np.stack([np.asarray(o)[:, :, :fw] for o in outs])  # [C, D, TPC, fw]
    return stack[g["gi_core"], g["gi_j"], g["gi_pos"], :].astype(np.float32)


def _run(nc, in_maps):
    res = run_bass_kernel_spmd(nc, in_maps, list(range(NCORES)))
    return res.results


class _Programs:
    def __init__(self, g):
        self.g = g
        self.p1 = _build_pass(g, "TP", WROW=2 * F_IN, FP=F_IN, HIOFF=F_IN,
                              Wo=2 * F_HID)
        self.p2 = _build_pass(g, "CB", WROW=2 * F_HID, FP=F_HID, HIOFF=F_HID,
                              KD=F_IN, relu=True)
        self.p3 = _build_pass(g, "TP", WROW=2 * F_HID, FP=F_HID, HIOFF=F_HID,
                              Wo=2 * F_OUT)
        self.p4 = _build_pass(g, "CB", WROW=2 * F_HID, FP=F_OUT, HIOFF=F_OUT,
                              KD=F_HID, relu=False, out_f32=True)


def kernel(x, edge_index, edge_weight, W1, b1, W2, b2):
    x = np.asarray(x, np.float32)
    edge_index = np.asarray(edge_index)
    edge_weight = np.asarray(edge_weight, np.float32)
    W1 = np.asarray(W1, np.float32)
    b1 = np.asarray(b1, np.float32)
    W2 = np.asarray(W2, np.float32)
    b2 = np.asarray(b2, np.float32)

    g = _prep_graph(edge_index, edge_weight)
    progs = _Programs(g)
    return _run_all(g, progs, x, W1, b1, W2, b2)


_IOTA = np.tile(np.arange(D, dtype=np.float32).astype(BF)[None, :], (P, 1))


def _core_inputs(g, c, tabs, extra):
    m = {"tab": tabs[c], "idx": g["idxw"][c], "meta": g["meta"][c],
         "iot": _IOTA}
    m.update(extra(c) if callable(extra) else extra)
    return m


def _run_all(g, progs, x, W1, b1, W2, b2):
    xb = _to_bf(x)
    nodelist = g["nodelist"]                    # [C, TPC, D] node or N
    xpad = np.zeros((N + 1, F_IN), BF)
    xpad[:N] = xb

    # P1: propagate x; per tile emit [y1 | t11] = agg^T [2W12 | W11] + [0|b1]
    w1cat = np.concatenate([2.0 * W1[2], W1[1]], axis=1)        # [128, 128]
    b1cat = np.concatenate([np.zeros(F_HID, np.float32), b1])[None, :]
    tabs = _build_tables(g, xb, 2 * F_IN, F_IN, F_IN)
    maps = [_core_inputs(g, c, tabs,
                         {"wcat": w1cat.astype(BF), "bcat": b1cat.astype(BF)})
            for c in range(NCORES)]
    res = _run(progs.p1, maps)
    out1 = [np.asarray(r["out"]) for r in res]          # [D, TPC, 128] bf16
    y1_full = _unpermute(g, out1, F_HID).astype(BF)     # 2(Lx)W12 rows

    # P2: h = relu(L y1 + x (W10-W12) + t11)
    w2d = (W1[0] - W1[2]).astype(BF)                     # [128, 64]
    idn = np.eye(D, dtype=np.float32).astype(BF)
    tabs = _build_tables(g, y1_full, 2 * F_HID, F_HID, F_HID)

    def p2_extra(c):
        xt = xpad[nodelist[c].reshape(-1)]               # [TPC*D, 128]
        return {"wd": w2d,
                "inT": np.ascontiguousarray(xt.T).astype(BF),
                "addin": np.ascontiguousarray(
                    out1[c][:, :, F_HID:]).astype(BF),
                "ident": idn}
    maps = [_core_inputs(g, c, tabs, p2_extra) for c in range(NCORES)]
    res = _run(progs.p2, maps)
    outh = [np.asarray(r["out"]) for r in res]           # [D, TPC, 64] bf16
    h_full = _unpermute(g, outh, F_HID).astype(BF)

    # P3: propagate h; per tile emit [y2 | u21b]
    w2cat = np.concatenate([2.0 * W2[2], W2[1]], axis=1)         # [64, 80]
    b2cat = np.concatenate([np.zeros(F_OUT, np.float32), b2])[None, :]
    tabs = _build_tables(g, h_full, 2 * F_HID, F_HID, F_HID)
    maps = [_core_inputs(g, c, tabs,
                         {"wcat": w2cat.astype(BF), "bcat": b2cat.astype(BF)})
            for c in range(NCORES)]
    res = _run(progs.p3, maps)
    out3 = [np.asarray(r["out"]) for r in res]           # [D, TPC, 80] bf16
    y2_full = _unpermute(g, out3, F_OUT).astype(BF)

    # P4: out = L y2 + h (W20-W22) + u21b
    w4d = (W2[0] - W2[2]).astype(BF)                     # [64, 40]
    hpad = np.zeros((N + 1, F_HID), BF)
    hpad[:N] = h_full
    tabs = _build_tables(g, y2_full, 2 * F_HID, F_OUT, F_OUT)

    def p4_extra(c):
        ht = hpad[nodelist[c].reshape(-1)]
        return {"wd": w4d,
                "inT": np.ascontiguousarray(ht.T).astype(BF),
                "addin": np.ascontiguousarray(
                    out3[c][:, :, F_OUT:]).astype(BF),
                "ident": idn}
    maps = [_core_inputs(g, c, tabs, p4_extra) for c in range(NCORES)]
    res = _run(progs.p4, maps)
    out4 = [np.asarray(r["out"]) for r in res]           # [D, TPC, 40] f32
    return _unpermute(g, out4, F_OUT)


# revision 4
# speedup vs baseline: 1.1464x; 1.0288x over previous
"""ChebNet (2-layer ChebConv, K=3) on 8 Trainium2 NeuronCores — v3.

Math: propagation commutes with feature matmuls, so per layer
    out = x(W0-W2) + (Lx)W1 + 2 L((Lx)W2) + b
giving 4 sparse propagations total (2 per layer) plus small dense matmuls
that are fused into the PSUM accumulation of each propagation pass:
  P1: agg1 = L x           -> per tile: [y1 | t11] = agg1^T [2W12 | W11] (+b1)
  P2: h = relu(L y1 + x(W10-W12) + t11)          (all summed in PSUM)
  P3: agg3 = L h           -> per tile: [y2 | u21] = agg3^T [2W22 | W21] (+b2)
  P4: out = L y2 + h(W20-W22) + u21              (all summed in PSUM)

Sharding: dest nodes are packed into T=1096 tiles of <=96 nodes, balanced
so every (tile, src-chunk) cell has <=~240 edge slots (2 blocks of 128).
Each of the 8 cores owns 137 tiles. Sources live in 4 chunks of 25024 rows
(int16 gather indices).

Descriptor pairing: gather tables hold PAIR rows [feat[a] | feat[b]] so a
single 256B/512B descriptor feeds two edges of the same (tile, chunk)
cell. A greedy matcher pairs ~74% of edges; paired slots fill block 0 of
each cell (2 selector builds + 2 matmuls), unpaired slots use only the
lo half (1 build + 1 matmul).

Device pipeline per pass: dma_gather pair rows -> DVE builds bf16
S[slot, destcol] = norm * (iota == ld) -> PE accumulates per-tile psum
(+ fused dense epilogue) -> Act engine writes tiles out.
"""
import numpy as np
import ml_dtypes
from contextlib import ExitStack

import concourse.bass as bass
import concourse.bacc as bacc
import concourse.mybir as mybir
import concourse.tile as tile
from concourse.bass_utils import run_bass_kernel_spmd

# problem constants
N = 100000
E = 1600000
F_IN = 128
F_HID = 64
F_OUT = 40

P = 128
D = 96                  # dest nodes per tile
NCORES = 8
TPC = 133               # tiles per core
T_TILES = TPC * NCORES  # 1096
NCHUNK = 4
CH = 25024              # source rows per chunk (int16 safe)
GRP = 4                 # tiles per psum group (acc banks + 2 psum2 banks)
PAIR_CAP = 128          # max paired slots per cell (keeps block0 pure)
RADJ = 3                # max adjacencies per source in the matcher
MAXROWS = 32700         # table row budget (int16)
SCRATCH = 16384         # swdge ring: 1024 descs per gather call

F32 = mybir.dt.float32
BF16 = mybir.dt.bfloat16
I16 = mybir.dt.int16
BF = ml_dtypes.bfloat16


# ---------------------------------------------------------------------------
# host-side graph preprocessing
# ---------------------------------------------------------------------------

def _pack_tiles(col, chunk):
    """Assign dest nodes to T_TILES tiles (<=D nodes each), balancing the
    per-(tile, chunk) edge counts. Greedy min-max over 4 chunk dims."""
    d = np.zeros((N, NCHUNK), np.int32)
    np.add.at(d, (col, chunk), 1)
    deg_tot = d.sum(1)
    order = np.argsort(-deg_tot, kind="stable")
    loads = np.zeros((T_TILES, NCHUNK), np.int32)
    counts = np.zeros(T_TILES, np.int32)
    assign = np.full(N, -1, np.int32)
    BIG = 1 << 20
    for v in order:
        cand = loads + d[v][None, :]
        m = cand.max(1).astype(np.int64)
        m[counts >= D] = BIG
        t = int(np.argmin(m))
        assign[v] = t
        loads[t] += d[v]
        counts[t] += 1
    return assign


def _match_core_chunk(s, cellid, ld, nrm, ncells):
    """Greedy pair matching for one (core, chunk).

    s: local source id per edge (0..CH-1); cellid: local tile per edge;
    ld/nrm: dest column and weight per edge.
    Emits explicit SLOTS: a paired slot carries two edges of one cell that
    share a table pair-row; a single slot carries one edge on the row's lo
    half. Returns dict with pairs[nrows,2] and per-slot arrays.
    """
    ne = len(s)
    o = np.argsort(cellid, kind="stable")
    s_s, eid_s = s[o], o
    bounds = np.searchsorted(cellid[o], np.arange(ncells + 1))
    adj_used = np.zeros(CH, np.int16)
    partners = {}            # src -> list of (partner, row_id, my_half)
    self_row = {}            # src -> row_id of (v,v) adjacency
    pairs = []
    sl_row, sl_cell, sl_pair = [], [], []
    sl_ldlo, sl_nrmlo, sl_ldhi, sl_nrmhi = [], [], [], []

    def emit_pair(rid, ci, e_lo, e_hi):
        sl_row.append(rid)
        sl_cell.append(ci)
        sl_pair.append(True)
        sl_ldlo.append(ld[e_lo])
        sl_nrmlo.append(nrm[e_lo])
        sl_ldhi.append(ld[e_hi])
        sl_nrmhi.append(nrm[e_hi])

    def emit_single(rid, ci, e):
        sl_row.append(rid)
        sl_cell.append(ci)
        sl_pair.append(False)
        sl_ldlo.append(ld[e])
        sl_nrmlo.append(nrm[e])
        sl_ldhi.append(-1.0)
        sl_nrmhi.append(0.0)

    unmatched_by_src = {}
    for ci in range(ncells):
        a0, b0 = bounds[ci], bounds[ci + 1]
        if a0 == b0:
            continue
        merged = 0
        open_e = {}
        for k in range(a0, b0):
            open_e.setdefault(int(s_s[k]), []).append(int(eid_s[k]))
        # 1. self pairs (two edges of same source in this cell share a row)
        for v, lst in open_e.items():
            while len(lst) >= 2 and merged < PAIR_CAP:
                rid = self_row.get(v, -1)
                if rid < 0:
                    if adj_used[v] <= RADJ - 2 and len(pairs) < MAXROWS:
                        adj_used[v] += 2
                        rid = len(pairs)
                        pairs.append((v, v))
                        self_row[v] = rid
                    else:
                        break
                emit_pair(rid, ci, lst.pop(), lst.pop())
                merged += 1
        # 2. reuse existing adjacencies
        for v in list(open_e.keys()):
            lst = open_e[v]
            if not lst or merged >= PAIR_CAP:
                continue
            for p, rid, half_v in partners.get(v, ()):
                if not lst or merged >= PAIR_CAP:
                    break
                plst = open_e.get(p)
                if p != v and plst:
                    ev, ep = lst.pop(), plst.pop()
                    if half_v == 0:
                        emit_pair(rid, ci, ev, ep)
                    else:
                        emit_pair(rid, ci, ep, ev)
                    merged += 1
        # 3. create new adjacencies among remaining
        flat = [(v, ei) for v, lst in open_e.items() for ei in lst]
        free, stuck = [], []
        for v, ei in flat:
            (free if adj_used[v] < RADJ and len(pairs) < MAXROWS
             else stuck).append((v, ei))
        while len(free) >= 2 and merged < PAIR_CAP and len(pairs) < MAXROWS:
            v1, e1 = free.pop()
            if free[-1][0] == v1:
                k = next((i for i in range(len(free)) if free[i][0] != v1), -1)
                if k < 0:
                    stuck.append((v1, e1))
                    stuck.extend(free)
                    free = []
                    break
                free[k], free[-1] = free[-1], free[k]
            v2, e2 = free.pop()
            adj_used[v1] += 1
            adj_used[v2] += 1
            rid = len(pairs)
            pairs.append((v1, v2))
            partners.setdefault(v1, []).append((v2, rid, 0))
            partners.setdefault(v2, []).append((v1, rid, 1))
            emit_pair(rid, ci, e1, e2)
            merged += 1
        stuck.extend(free)
        for v, ei in stuck:
            unmatched_by_src.setdefault(v, []).append((ci, ei))

    # unmatched edges: single slots on a (v,v) row's lo half
    for v, lst in unmatched_by_src.items():
        rid = self_row.get(v, -1)
        if rid < 0:
            rid = len(pairs)
            pairs.append((v, v))
            self_row[v] = rid
        for ci, ei in lst:
            emit_single(rid, ci, ei)
    assert len(pairs) <= 32767, len(pairs)
    nsl = len(sl_row)
    assert nsl == 0 or 2 * sum(sl_pair) + (nsl - sum(sl_pair)) == ne
    return dict(
        pairs=np.array(pairs, np.int64).reshape(-1, 2),
        srow=np.array(sl_row, np.int64),
        scell=np.array(sl_cell, np.int64),
        spair=np.array(sl_pair, bool),
        ldlo=np.array(sl_ldlo, np.float32),
        nrmlo=np.array(sl_nrmlo, np.float32),
        ldhi=np.array(sl_ldhi, np.float32),
        nrmhi=np.array(sl_nrmhi, np.float32),
    )


def _prep_graph(edge_index, edge_weight):
    row = np.ascontiguousarray(edge_index[0]).astype(np.int64)
    col = np.ascontiguousarray(edge_index[1]).astype(np.int64)
    w = np.ascontiguousarray(edge_weight).astype(np.float32)

    deg = np.bincount(row, weights=w.astype(np.float64), minlength=N).astype(np.float32)
    dinv = np.where(deg > 0, 1.0 / np.sqrt(np.maximum(deg, 1e-30)), 0.0).astype(np.float32)
    norm = (-dinv[row] * w * dinv[col]).astype(np.float32)

    chunk = np.minimum(row // CH, NCHUNK - 1)
    assign = _pack_tiles(col, chunk)

    # dest-local column within tile; node <-> (tile, col) maps
    order = np.argsort(assign, kind="stable")
    t_sorted = assign[order]
    start = np.searchsorted(t_sorted, np.arange(T_TILES + 1))
    ldcol = np.zeros(N, np.int64)
    tile_nodes = np.full((T_TILES, D), -1, np.int64)
    for t in range(T_TILES):
        nodes = order[start[t]:start[t + 1]]
        ldcol[nodes] = np.arange(len(nodes))
        tile_nodes[t, :len(nodes)] = nodes

    tile_of_edge = assign[col]
    core_of_edge = tile_of_edge // TPC
    ld_of_edge = ldcol[col]

    # per (core, chunk) matching
    cores = []
    NR = 0
    nslots_all = np.zeros((NCORES, TPC, NCHUNK), np.int64)
    for c in range(NCORES):
        csel = np.nonzero(core_of_edge == c)[0]
        per_chunk = []
        for ch in range(NCHUNK):
            sel = csel[chunk[csel] == ch]
            s_local = (row[sel] - ch * CH).astype(np.int64)
            cellid = (tile_of_edge[sel] - c * TPC).astype(np.int64)
            st = _match_core_chunk(s_local, cellid, ld_of_edge[sel],
                                   norm[sel], TPC)
            per_chunk.append(st)
            NR = max(NR, len(st["pairs"]))
            nslots_all[c, :, ch] = np.bincount(st["scell"], minlength=TPC)
        cores.append(per_chunk)
    nb_all = np.maximum(1, -(-nslots_all // P))

    # rank-align tiles across cores: sort each core's tiles by total nb
    tile_perm = np.zeros((NCORES, TPC), np.int64)
    for c in range(NCORES):
        tile_perm[c] = np.lexsort((np.arange(TPC), -nb_all[c].sum(1)))
    nb_sorted = np.stack([nb_all[c][tile_perm[c]] for c in range(NCORES)])
    NB = nb_sorted.max(0)                     # [TPC(pos), NCHUNK]
    B_TOTAL = int(NB.sum())
    SLOTS = B_TOTAL * P

    # shared block table + call plan (GRP positions, chunk-major runs).
    # hardware SWDGE ring holds 1024 descriptors -> gather calls <= 8 blocks.
    MAXBLK = SCRATCH // (16 * P)
    block_pos = []
    block_ch = []
    block_bi = []
    calls = []                                # (block0, nblocks, ch)
    groups = []                               # (pos0, npos)
    b = 0
    for g0 in range(0, TPC, GRP):
        gn = min(GRP, TPC - g0)
        groups.append((g0, gn))
        for ch in range(NCHUNK):
            nbk = int(NB[g0:g0 + gn, ch].sum())
            sub = []
            o = 0
            while o < nbk:
                n1 = min(MAXBLK, nbk - o)
                sub.append((b + o, n1))
                o += n1
            calls.append((sub, ch))
            for pos in range(g0, g0 + gn):
                for bi in range(int(NB[pos, ch])):
                    block_pos.append(pos)
                    block_ch.append(ch)
                    block_bi.append(bi)
                    b += 1
    assert b == B_TOTAL

    # per-core slot arrays
    blk_of = {}
    for bb in range(B_TOTAL):
        blk_of.setdefault((block_pos[bb], block_ch[bb]), []).append(bb)

    idx16 = np.zeros((NCORES, SLOTS), np.int16)
    meta = np.zeros((NCORES, P, 4 * B_TOTAL), np.float32)
    meta[:, :, 0::4] = -1.0                   # ld_lo
    meta[:, :, 2::4] = -1.0                   # ld_hi
    first_blk = {k: v[0] for k, v in blk_of.items()}
    for c in range(NCORES):
        inv_pos = np.zeros(TPC, np.int64)
        inv_pos[tile_perm[c]] = np.arange(TPC)
        for ch in range(NCHUNK):
            st = cores[c][ch]
            nslot = len(st["srow"])
            if nslot == 0:
                continue
            # order: by cell, paired slots first within cell
            ordr = np.lexsort((np.arange(nslot), ~st["spair"], st["scell"]))
            cell_s = st["scell"][ordr]
            pair_s = st["spair"][ordr]
            # global slot position for each ordered slot
            cb = np.searchsorted(cell_s, np.arange(TPC + 1))
            within = np.arange(nslot) - cb[cell_s]
            base_blk = np.array([first_blk[(int(inv_pos[tl]), ch)]
                                 for tl in range(TPC)], np.int64)
            npaired_cell = np.bincount(cell_s[pair_s], minlength=TPC)
            assert npaired_cell.max() <= P, npaired_cell.max()
            blk = base_blk[cell_s] + within // P
            prt = within % P
            slot = blk * P + prt
            idx16[c, slot] = st["srow"][ordr].astype(np.int16)
            meta[c, prt, 4 * blk] = st["ldlo"][ordr]
            meta[c, prt, 4 * blk + 1] = st["nrmlo"][ordr]
            meta[c, prt, 4 * blk + 2] = st["ldhi"][ordr]
            meta[c, prt, 4 * blk + 3] = st["nrmhi"][ordr]

    # wrapped idx layout [16, SLOTS/16] tiled to 128 partitions
    ii = np.arange(SLOTS)
    idxw = np.zeros((NCORES, 16, SLOTS // 16), np.int16)
    idxw[:, ii % 16, ii // 16] = idx16
    idxw = np.tile(idxw, (1, 8, 1))

    # node <-> (core, pos, ldcol) output mapping
    pos_of_tile = np.zeros(T_TILES, np.int64)
    for c in range(NCORES):
        pos_of_tile[tile_perm[c] + c * TPC] = np.arange(TPC)
    vnodes = np.arange(N)
    gi_core = assign[vnodes] // TPC
    gi_pos = pos_of_tile[assign[vnodes]]
    gi_j = ldcol[vnodes]
    # per-core pos-major node list [TPC, D] (entry: node id or N for pad)
    nodelist = np.full((NCORES, TPC, D), N, np.int64)
    for c in range(NCORES):
        tl = tile_perm[c] + c * TPC
        tn = tile_nodes[tl]                  # [TPC, D]
        nodelist[c] = np.where(tn >= 0, tn, N)

    return dict(cores=cores, NR=NR, NB=NB, B_TOTAL=B_TOTAL, SLOTS=SLOTS,
                calls=calls, groups=groups,
                block_pos=np.array(block_pos), block_ch=np.array(block_ch),
                block_bi=np.array(block_bi),
                idxw=idxw, meta=meta, idx16=idx16,
                gi_core=gi_core, gi_pos=gi_pos, gi_j=gi_j,
                nodelist=nodelist)


# ---------------------------------------------------------------------------
# device program builder
# ---------------------------------------------------------------------------

def _build_pass(g, mode, WROW, FP, HIOFF, Wo=None, KD=None, relu=False,
                out_f32=False):
    """One propagation pass.

    mode 'TP': psum acc [FP, D] (transposed); epilogue out = acc^T @ wcat
               (+ ones @ bcat) -> [D, Wo] tiles.
    mode 'CB': psum acc [D, FP]; epilogue acc += inT_tile^T @ wd
               + ident @ addin_tile; out = act(acc) -> [D, FP] tiles.
    """
    NR, NB, B_TOTAL, SLOTS = g["NR"], g["NB"], g["B_TOTAL"], g["SLOTS"]
    calls, groups = g["calls"], g["groups"]
    block_pos, block_ch, block_bi = g["block_pos"], g["block_ch"], g["block_bi"]

    nc = bacc.Bacc("TRN2", target_bir_lowering=False,
                   dynamic_dma_scratch_size=SCRATCH)
    tab = nc.declare_dram_parameter("tab", [NCHUNK * NR, WROW], BF16, isOutput=False)
    idx = nc.declare_dram_parameter("idx", [P, SLOTS // 16], I16, isOutput=False)
    meta = nc.declare_dram_parameter("meta", [P, 4 * B_TOTAL], F32, isOutput=False)
    iot = nc.declare_dram_parameter("iot", [P, D], BF16, isOutput=False)
    if mode == "TP":
        wcat = nc.declare_dram_parameter("wcat", [FP, Wo], BF16, isOutput=False)
        bcat = nc.declare_dram_parameter("bcat", [1, Wo], BF16, isOutput=False)
        out = nc.declare_dram_parameter("out", [D, TPC, Wo], BF16, isOutput=True)
        WO = Wo
    else:
        wd = nc.declare_dram_parameter("wd", [KD, FP], BF16, isOutput=False)
        inT = nc.declare_dram_parameter("inT", [KD, TPC * D], BF16, isOutput=False)
        addin = nc.declare_dram_parameter("addin", [D, TPC, FP], BF16, isOutput=False)
        ident = nc.declare_dram_parameter("ident", [D, D], BF16, isOutput=False)
        odt = F32 if out_f32 else BF16
        out = nc.declare_dram_parameter("out", [D, TPC, FP], odt, isOutput=True)
        WO = FP

    with ExitStack() as ctx:
        tc = ctx.enter_context(tile.TileContext(nc))
        cpool = ctx.enter_context(tc.tile_pool(name="const", bufs=1))
        gpool = ctx.enter_context(tc.tile_pool(name="g", bufs=8))
        spool = ctx.enter_context(tc.tile_pool(name="s", bufs=24))
        sbpool = ctx.enter_context(tc.tile_pool(name="sb", bufs=4))
        stpool = ctx.enter_context(tc.tile_pool(name="st", bufs=3))
        apool = ctx.enter_context(tc.tile_pool(name="acc", bufs=GRP, space="PSUM"))
        if mode == "TP":
            p2pool = ctx.enter_context(tc.tile_pool(name="p2", bufs=2, space="PSUM"))

        idx_t = cpool.tile([P, SLOTS // 16], I16)
        meta_t = cpool.tile([P, 4 * B_TOTAL], F32)
        IH = (SLOTS // 16) // 2
        MH = (4 * B_TOTAL) // 2
        nc.sync.dma_start(out=idx_t[:, :IH], in_=idx[:, :IH])
        nc.sync.dma_start(out=meta_t[:, :MH], in_=meta[:, :MH])
        nc.sync.dma_start(out=idx_t[:, IH:], in_=idx[:, IH:])
        nc.sync.dma_start(out=meta_t[:, MH:], in_=meta[:, MH:])
        iota_b = cpool.tile([P, D], BF16)
        nc.sync.dma_start(out=iota_b[:], in_=iot[:])
        if mode == "TP":
            wcat_t = cpool.tile([FP, Wo], BF16)
            bcat_t = cpool.tile([1, Wo], BF16)
            ones_t = cpool.tile([1, D], BF16)
            nc.sync.dma_start(out=wcat_t[:], in_=wcat[:])
            nc.sync.dma_start(out=bcat_t[:], in_=bcat[:])
            nc.vector.memset(ones_t[:], 1.0)
        else:
            wd_t = cpool.tile([KD, FP], BF16)
            ident_t = cpool.tile([D, D], BF16)
            nc.sync.dma_start(out=wd_t[:], in_=wd[:])
            nc.sync.dma_start(out=ident_t[:], in_=ident[:])
            inpool = ctx.enter_context(tc.tile_pool(name="inp", bufs=3))
            adpool = ctx.enter_context(tc.tile_pool(name="adp", bufs=3))

        acc = {}
        ci = 0
        for (g0, gn) in groups:
            if mode == "CB":
                int_g = inpool.tile([KD, GRP * D], BF16, tag="inp")
                ad_g = adpool.tile([D, GRP, FP], BF16, tag="adp")
                nc.sync.dma_start(out=int_g[:, :gn * D],
                                  in_=inT[:, g0 * D:(g0 + gn) * D])
                nc.sync.dma_start(out=ad_g[:, :gn, :],
                                  in_=addin[:, g0:g0 + gn, :])
            for ch in range(NCHUNK):
                sub, _ch = calls[ci]
                ci += 1
                for (b0, nbk) in sub:
                    gt = gpool.tile([P, nbk, WROW], BF16, tag="g")
                    nc.gpsimd.dma_gather(
                        gt[:], tab[ch * NR:(ch + 1) * NR, :],
                        idx_t[:, b0 * 8:(b0 + nbk) * 8],
                        nbk * P, nbk * P, WROW,
                    )
                    for j in range(nbk):
                        bb = b0 + j
                    pos = int(block_pos[bb])
                    bi = int(block_bi[bb])
                    first = (ch == 0 and bi == 0)
                    last_s = (ch == NCHUNK - 1 and bi == int(NB[pos, ch]) - 1)
                    if first:
                        shape = [FP, D] if mode == "TP" else [D, FP]
                        acc[pos] = apool.tile(shape, F32, space="PSUM",
                                              tag="acc", name="acc")
                    mcol = 4 * bb
                    S_lo = spool.tile([P, D], BF16, tag="S")
                    nc.vector.tensor_scalar(
                        out=S_lo[:], in0=iota_b[:],
                        scalar1=meta_t[:, mcol:mcol + 1],
                        scalar2=meta_t[:, mcol + 1:mcol + 2],
                        op0=mybir.AluOpType.is_equal,
                        op1=mybir.AluOpType.mult,
                    )
                    # hi selector only on block 0 (paired slots live there)
                    do_hi = (bi == 0)
                    if mode == "TP":
                        nc.tensor.matmul(out=acc[pos][:], lhsT=gt[:, j, 0:FP],
                                         rhs=S_lo[:], start=first,
                                         stop=(mode == "TP" and last_s and not do_hi))
                    else:
                        nc.tensor.matmul(out=acc[pos][:], lhsT=S_lo[:],
                                         rhs=gt[:, j, 0:FP], start=first,
                                         stop=False)
                    if do_hi:
                        S_hi = spool.tile([P, D], BF16, tag="S")
                        nc.vector.tensor_scalar(
                            out=S_hi[:], in0=iota_b[:],
                            scalar1=meta_t[:, mcol + 2:mcol + 3],
                            scalar2=meta_t[:, mcol + 3:mcol + 4],
                            op0=mybir.AluOpType.is_equal,
                            op1=mybir.AluOpType.mult,
                        )
                        if mode == "TP":
                            nc.tensor.matmul(out=acc[pos][:],
                                             lhsT=gt[:, j, HIOFF:HIOFF + FP],
                                             rhs=S_hi[:], start=False,
                                             stop=last_s)
                        else:
                            nc.tensor.matmul(out=acc[pos][:], lhsT=S_hi[:],
                                             rhs=gt[:, j, HIOFF:HIOFF + FP],
                                             start=False, stop=False)
            # group epilogue
            st = stpool.tile([D, GRP, WO], F32 if (mode == "CB" and out_f32) else BF16, tag="st")
            for k in range(gn):
                pos = g0 + k
                if mode == "TP":
                    sb = sbpool.tile([FP, D], BF16, tag="sb")
                    nc.scalar.activation(sb[:], acc[pos][:],
                                         mybir.ActivationFunctionType.Copy)
                    p2 = p2pool.tile([D, Wo], F32, space="PSUM", tag="p2")
                    nc.tensor.matmul(out=p2[:], lhsT=sb[:], rhs=wcat_t[:],
                                     start=True, stop=False)
                    nc.tensor.matmul(out=p2[:], lhsT=ones_t[:], rhs=bcat_t[:],
                                     start=False, stop=True)
                    nc.scalar.activation(st[:, k, :], p2[:],
                                         mybir.ActivationFunctionType.Copy)
                else:
                    nc.tensor.matmul(out=acc[pos][:],
                                     lhsT=int_g[:, k * D:(k + 1) * D],
                                     rhs=wd_t[:], start=False, stop=False)
                    nc.tensor.matmul(out=acc[pos][:], lhsT=ident_t[:],
                                     rhs=ad_g[:, k, :],
                                     start=False, stop=True)
                    fn = (mybir.ActivationFunctionType.Relu if relu
                          else mybir.ActivationFunctionType.Copy)
                    nc.scalar.activation(st[:, k, :], acc[pos][:], fn)
                del acc[pos]
            nc.sync.dma_start(out=out[:, g0:g0 + gn, :], in_=st[:, :gn, :])

    nc.compile()
    return nc


# ---------------------------------------------------------------------------
# host glue
# ---------------------------------------------------------------------------

def _to_bf(a):
    return np.asarray(a, np.float32).astype(BF)


def _build_tables(g, feat, wrow, fp, hioff):
    """Per-core gather tables [NCHUNK*NR, wrow] from full features
    feat [N, fw] (fw = feat width <= fp slots in the row)."""
    NR = g["NR"]
    fw = feat.shape[1]
    featp = np.zeros((NCHUNK * CH, fw), BF)
    featp[:N] = feat
    tabs = []
    for c in range(NCORES):
        t = np.zeros((NCHUNK * NR, wrow), BF)
        for ch in range(NCHUNK):
            pairs = g["cores"][c][ch]["pairs"]
            nr = len(pairs)
            if nr == 0:
                continue
            base = featp[ch * CH:(ch + 1) * CH]
            t[ch * NR:ch * NR + nr, 0:fw] = base[pairs[:, 0]]
            t[ch * NR:ch * NR + nr, hioff:hioff + fw] = base[pairs[:, 1]]
        tabs.append(t)
    return tabs


def _unpermute(g, outs, fw):
    """outs: list of 8 arrays [D, TPC, >=fw] -> full [N, fw] float32."""
    stack = np.stack([np.asarray(o)[:, :, :fw] for o in outs])  # [C, D, TPC, fw]
    return stack[g["gi_core"], g["gi_j"], g["gi_pos"], :].astype(np.float32)


def _run(nc, in_maps):
    res = run_bass_kernel_spmd(nc, in_maps, list(range(NCORES)))
    return res.results


class _Programs:
    def __init__(self, g):
        self.g = g
        self.p1 = _build_pass(g, "TP", WROW=2 * F_IN, FP=F_IN, HIOFF=F_IN,
                              Wo=2 * F_HID)
        self.p2 = _build_pass(g, "CB", WROW=2 * F_HID, FP=F_HID, HIOFF=F_HID,
                              KD=F_IN, relu=True)
        self.p3 = _build_pass(g, "TP", WROW=2 * F_HID, FP=F_HID, HIOFF=F_HID,
                              Wo=2 * F_OUT)
        self.p4 = _build_pass(g, "CB", WROW=2 * F_HID, FP=F_OUT, HIOFF=F_OUT,
                              KD=F_HID, relu=False, out_f32=True)


def kernel(x, edge_index, edge_weight, W1, b1, W2, b2):
    x = np.asarray(x, np.float32)
    edge_index = np.asarray(edge_index)
    edge_weight = np.asarray(edge_weight, np.float32)
    W1 = np.asarray(W1, np.float32)
    b1 = np.asarray(b1, np.float32)
    W2 = np.asarray(W2, np.float32)
    b2 = np.asarray(b2, np.float32)

    g = _prep_graph(edge_index, edge_weight)
    progs = _Programs(g)
    return _run_all(g, progs, x, W1, b1, W2, b2)


_IOTA = np.tile(np.arange(D, dtype=np.float32).astype(BF)[None, :], (P, 1))


def _core_inputs(g, c, tabs, extra):
    m = {"tab": tabs[c], "idx": g["idxw"][c], "meta": g["meta"][c],
         "iot": _IOTA}
    m.update(extra(c) if callable(extra) else extra)
    return m


def _run_all(g, progs, x, W1, b1, W2, b2):
    xb = _to_bf(x)
    nodelist = g["nodelist"]                    # [C, TPC, D] node or N
    xpad = np.zeros((N + 1, F_IN), BF)
    xpad[:N] = xb

    # P1: propagate x; per tile emit [y1 | t11] = agg^T [2W12 | W11] + [0|b1]
    w1cat = np.concatenate([2.0 * W1[2], W1[1]], axis=1)        # [128, 128]
    b1cat = np.concatenate([np.zeros(F_HID, np.float32), b1])[None, :]
    tabs = _build_tables(g, xb, 2 * F_IN, F_IN, F_IN)
    maps = [_core_inputs(g, c, tabs,
                         {"wcat": w1cat.astype(BF), "bcat": b1cat.astype(BF)})
            for c in range(NCORES)]
    res = _run(progs.p1, maps)
    out1 = [np.asarray(r["out"]) for r in res]          # [D, TPC, 128] bf16
    y1_full = _unpermute(g, out1, F_HID).astype(BF)     # 2(Lx)W12 rows

    # P2: h = relu(L y1 + x (W10-W12) + t11)
    w2d = (W1[0] - W1[2]).astype(BF)                     # [128, 64]
    idn = np.eye(D, dtype=np.float32).astype(BF)
    tabs = _build_tables(g, y1_full, 2 * F_HID, F_HID, F_HID)

    def p2_extra(c):
        xt = xpad[nodelist[c].reshape(-1)]               # [TPC*D, 128]
        return {"wd": w2d,
                "inT": np.ascontiguousarray(xt.T).astype(BF),
                "addin": np.ascontiguousarray(
                    out1[c][:, :, F_HID:]).astype(BF),
                "ident": idn}
    maps = [_core_inputs(g, c, tabs, p2_extra) for c in range(NCORES)]
    res = _run(progs.p2, maps)
    outh = [np.asarray(r["out"]) for r in res]           # [D, TPC, 64] bf16
    h_full = _unpermute(g, outh, F_HID).astype(BF)

    # P3: propagate h; per tile emit [y2 | u21b]
    w2cat = np.concatenate([2.0 * W2[2], W2[1]], axis=1)         # [64, 80]
    b2cat = np.concatenate([np.zeros(F_OUT, np.float32), b2])[None, :]
    tabs = _build_tables(g, h_full, 2 * F_HID, F_HID, F_HID)
    maps = [_core_inputs(g, c, tabs,
                         {"wcat": w2cat.astype(BF), "bcat": b2cat.astype(BF)})
            for c in range(NCORES)]
    res = _run(progs.p3, maps)
    out3 = [np.asarray(r["out"]) for r in res]           # [D, TPC, 80] bf16
    y2_full = _unpermute(g, out3, F_OUT).astype(BF)

    # P4: out = L y2 + h (W20-W22) + u21b
    w4d = (W2[0] - W2[2]).astype(BF)                     # [64, 40]
    hpad = np.zeros((N + 1, F_HID), BF)
    hpad[:N] = h_full
    tabs = _build_tables(g, y2_full, 2 * F_HID, F_OUT, F_OUT)

    def p4_extra(c):
        ht = hpad[nodelist[c].reshape(-1)]
        return {"wd": w4d,
                "inT": np.ascontiguousarray(ht.T).astype(BF),
                "addin": np.ascontiguousarray(
                    out3[c][:, :, F_OUT:]).astype(BF),
                "ident": idn}
    maps = [_core_inputs(g, c, tabs, p4_extra) for c in range(NCORES)]
    res = _run(progs.p4, maps)
    out4 = [np.asarray(r["out"]) for r in res]           # [D, TPC, 40] f32
    return _unpermute(g, out4, F_OUT)


# revision 5
# speedup vs baseline: 1.1566x; 1.0089x over previous
"""ChebNet (2-layer ChebConv, K=3) on 8 Trainium2 NeuronCores — v3.

Math: propagation commutes with feature matmuls, so per layer
    out = x(W0-W2) + (Lx)W1 + 2 L((Lx)W2) + b
giving 4 sparse propagations total (2 per layer) plus small dense matmuls
that are fused into the PSUM accumulation of each propagation pass:
  P1: agg1 = L x           -> per tile: [y1 | t11] = agg1^T [2W12 | W11] (+b1)
  P2: h = relu(L y1 + x(W10-W12) + t11)          (all summed in PSUM)
  P3: agg3 = L h           -> per tile: [y2 | u21] = agg3^T [2W22 | W21] (+b2)
  P4: out = L y2 + h(W20-W22) + u21              (all summed in PSUM)

Sharding: dest nodes are packed into T=1096 tiles of <=96 nodes, balanced
so every (tile, src-chunk) cell has <=~240 edge slots (2 blocks of 128).
Each of the 8 cores owns 137 tiles. Sources live in 4 chunks of 25024 rows
(int16 gather indices).

Descriptor pairing: gather tables hold PAIR rows [feat[a] | feat[b]] so a
single 256B/512B descriptor feeds two edges of the same (tile, chunk)
cell. A greedy matcher pairs ~74% of edges; paired slots fill block 0 of
each cell (2 selector builds + 2 matmuls), unpaired slots use only the
lo half (1 build + 1 matmul).

Device pipeline per pass: dma_gather pair rows -> DVE builds bf16
S[slot, destcol] = norm * (iota == ld) -> PE accumulates per-tile psum
(+ fused dense epilogue) -> Act engine writes tiles out.
"""
import numpy as np
import ml_dtypes
from contextlib import ExitStack

import concourse.bass as bass
import concourse.bacc as bacc
import concourse.mybir as mybir
import concourse.tile as tile
from concourse.bass_utils import run_bass_kernel_spmd

# problem constants
N = 100000
E = 1600000
F_IN = 128
F_HID = 64
F_OUT = 40

P = 128
D = 96                  # dest nodes per tile
NCORES = 8
TPC = 133               # tiles per core
T_TILES = TPC * NCORES  # 1096
NCHUNK = 4
CH = 25024              # source rows per chunk (int16 safe)
GRP = 4                 # tiles per psum group (acc banks + 2 psum2 banks)
PAIR_CAP = 128          # max paired slots per cell (keeps block0 pure)
RADJ = 3                # max adjacencies per source in the matcher
MAXROWS = 32700         # table row budget (int16)
SCRATCH = 16384         # swdge ring: 1024 descs per gather call

F32 = mybir.dt.float32
BF16 = mybir.dt.bfloat16
I16 = mybir.dt.int16
BF = ml_dtypes.bfloat16


# ---------------------------------------------------------------------------
# host-side graph preprocessing
# ---------------------------------------------------------------------------

def _pack_tiles(col, chunk):
    """Assign dest nodes to T_TILES tiles (<=D nodes each), balancing the
    per-(tile, chunk) edge counts. Greedy min-max over 4 chunk dims."""
    d = np.zeros((N, NCHUNK), np.int32)
    np.add.at(d, (col, chunk), 1)
    deg_tot = d.sum(1)
    order = np.argsort(-deg_tot, kind="stable")
    loads = np.zeros((T_TILES, NCHUNK), np.int32)
    counts = np.zeros(T_TILES, np.int32)
    assign = np.full(N, -1, np.int32)
    BIG = 1 << 20
    for v in order:
        cand = loads + d[v][None, :]
        m = cand.max(1).astype(np.int64)
        m[counts >= D] = BIG
        t = int(np.argmin(m))
        assign[v] = t
        loads[t] += d[v]
        counts[t] += 1
    return assign


def _match_core_chunk(s, cellid, ld, nrm, ncells):
    """Greedy pair matching for one (core, chunk).

    s: local source id per edge (0..CH-1); cellid: local tile per edge;
    ld/nrm: dest column and weight per edge.
    Emits explicit SLOTS: a paired slot carries two edges of one cell that
    share a table pair-row; a single slot carries one edge on the row's lo
    half. Returns dict with pairs[nrows,2] and per-slot arrays.
    """
    ne = len(s)
    o = np.argsort(cellid, kind="stable")
    s_s, eid_s = s[o], o
    bounds = np.searchsorted(cellid[o], np.arange(ncells + 1))
    adj_used = np.zeros(CH, np.int16)
    partners = {}            # src -> list of (partner, row_id, my_half)
    self_row = {}            # src -> row_id of (v,v) adjacency
    pairs = []
    sl_row, sl_cell, sl_pair = [], [], []
    sl_ldlo, sl_nrmlo, sl_ldhi, sl_nrmhi = [], [], [], []

    def emit_pair(rid, ci, e_lo, e_hi):
        sl_row.append(rid)
        sl_cell.append(ci)
        sl_pair.append(True)
        sl_ldlo.append(ld[e_lo])
        sl_nrmlo.append(nrm[e_lo])
        sl_ldhi.append(ld[e_hi])
        sl_nrmhi.append(nrm[e_hi])

    def emit_single(rid, ci, e):
        sl_row.append(rid)
        sl_cell.append(ci)
        sl_pair.append(False)
        sl_ldlo.append(ld[e])
        sl_nrmlo.append(nrm[e])
        sl_ldhi.append(-1.0)
        sl_nrmhi.append(0.0)

    unmatched_by_src = {}
    for ci in range(ncells):
        a0, b0 = bounds[ci], bounds[ci + 1]
        if a0 == b0:
            continue
        merged = 0
        open_e = {}
        for k in range(a0, b0):
            open_e.setdefault(int(s_s[k]), []).append(int(eid_s[k]))
        # 1. self pairs (two edges of same source in this cell share a row)
        for v, lst in open_e.items():
            while len(lst) >= 2 and merged < PAIR_CAP:
                rid = self_row.get(v, -1)
                if rid < 0:
                    if adj_used[v] <= RADJ - 2 and len(pairs) < MAXROWS:
                        adj_used[v] += 2
                        rid = len(pairs)
                        pairs.append((v, v))
                        self_row[v] = rid
                    else:
                        break
                emit_pair(rid, ci, lst.pop(), lst.pop())
                merged += 1
        # 2. reuse existing adjacencies
        for v in list(open_e.keys()):
            lst = open_e[v]
            if not lst or merged >= PAIR_CAP:
                continue
            for p, rid, half_v in partners.get(v, ()):
                if not lst or merged >= PAIR_CAP:
                    break
                plst = open_e.get(p)
                if p != v and plst:
                    ev, ep = lst.pop(), plst.pop()
                    if half_v == 0:
                        emit_pair(rid, ci, ev, ep)
                    else:
                        emit_pair(rid, ci, ep, ev)
                    merged += 1
        # 3. create new adjacencies among remaining
        flat = [(v, ei) for v, lst in open_e.items() for ei in lst]
        free, stuck = [], []
        for v, ei in flat:
            (free if adj_used[v] < RADJ and len(pairs) < MAXROWS
             else stuck).append((v, ei))
        while len(free) >= 2 and merged < PAIR_CAP and len(pairs) < MAXROWS:
            v1, e1 = free.pop()
            if free[-1][0] == v1:
                k = next((i for i in range(len(free)) if free[i][0] != v1), -1)
                if k < 0:
                    stuck.append((v1, e1))
                    stuck.extend(free)
                    free = []
                    break
                free[k], free[-1] = free[-1], free[k]
            v2, e2 = free.pop()
            adj_used[v1] += 1
            adj_used[v2] += 1
            rid = len(pairs)
            pairs.append((v1, v2))
            partners.setdefault(v1, []).append((v2, rid, 0))
            partners.setdefault(v2, []).append((v1, rid, 1))
            emit_pair(rid, ci, e1, e2)
            merged += 1
        stuck.extend(free)
        for v, ei in stuck:
            unmatched_by_src.setdefault(v, []).append((ci, ei))

    # unmatched edges: single slots on a (v,v) row's lo half
    for v, lst in unmatched_by_src.items():
        rid = self_row.get(v, -1)
        if rid < 0:
            rid = len(pairs)
            pairs.append((v, v))
            self_row[v] = rid
        for ci, ei in lst:
            emit_single(rid, ci, ei)
    assert len(pairs) <= 32767, len(pairs)
    nsl = len(sl_row)
    assert nsl == 0 or 2 * sum(sl_pair) + (nsl - sum(sl_pair)) == ne
    return dict(
        pairs=np.array(pairs, np.int64).reshape(-1, 2),
        srow=np.array(sl_row, np.int64),
        scell=np.array(sl_cell, np.int64),
        spair=np.array(sl_pair, bool),
        ldlo=np.array(sl_ldlo, np.float32),
        nrmlo=np.array(sl_nrmlo, np.float32),
        ldhi=np.array(sl_ldhi, np.float32),
        nrmhi=np.array(sl_nrmhi, np.float32),
    )


def _prep_graph(edge_index, edge_weight):
    row = np.ascontiguousarray(edge_index[0]).astype(np.int64)
    col = np.ascontiguousarray(edge_index[1]).astype(np.int64)
    w = np.ascontiguousarray(edge_weight).astype(np.float32)

    deg = np.bincount(row, weights=w.astype(np.float64), minlength=N).astype(np.float32)
    dinv = np.where(deg > 0, 1.0 / np.sqrt(np.maximum(deg, 1e-30)), 0.0).astype(np.float32)
    norm = (-dinv[row] * w * dinv[col]).astype(np.float32)

    chunk = np.minimum(row // CH, NCHUNK - 1)
    assign = _pack_tiles(col, chunk)

    # dest-local column within tile; node <-> (tile, col) maps
    order = np.argsort(assign, kind="stable")
    t_sorted = assign[order]
    start = np.searchsorted(t_sorted, np.arange(T_TILES + 1))
    ldcol = np.zeros(N, np.int64)
    tile_nodes = np.full((T_TILES, D), -1, np.int64)
    for t in range(T_TILES):
        nodes = order[start[t]:start[t + 1]]
        ldcol[nodes] = np.arange(len(nodes))
        tile_nodes[t, :len(nodes)] = nodes

    tile_of_edge = assign[col]
    core_of_edge = tile_of_edge // TPC
    ld_of_edge = ldcol[col]

    # per (core, chunk) matching
    cores = []
    NR = 0
    nslots_all = np.zeros((NCORES, TPC, NCHUNK), np.int64)
    for c in range(NCORES):
        csel = np.nonzero(core_of_edge == c)[0]
        per_chunk = []
        for ch in range(NCHUNK):
            sel = csel[chunk[csel] == ch]
            s_local = (row[sel] - ch * CH).astype(np.int64)
            cellid = (tile_of_edge[sel] - c * TPC).astype(np.int64)
            st = _match_core_chunk(s_local, cellid, ld_of_edge[sel],
                                   norm[sel], TPC)
            per_chunk.append(st)
            NR = max(NR, len(st["pairs"]))
            nslots_all[c, :, ch] = np.bincount(st["scell"], minlength=TPC)
        cores.append(per_chunk)
    nb_all = np.maximum(1, -(-nslots_all // P))

    # rank-align tiles across cores: sort each core's tiles by total nb
    tile_perm = np.zeros((NCORES, TPC), np.int64)
    for c in range(NCORES):
        tile_perm[c] = np.lexsort((np.arange(TPC), -nb_all[c].sum(1)))
    nb_sorted = np.stack([nb_all[c][tile_perm[c]] for c in range(NCORES)])
    NB = nb_sorted.max(0)                     # [TPC(pos), NCHUNK]
    B_TOTAL = int(NB.sum())
    SLOTS = B_TOTAL * P

    # shared block table + call plan (GRP positions, chunk-major runs).
    # hardware SWDGE ring holds 1024 descriptors -> gather calls <= 8 blocks.
    MAXBLK = SCRATCH // (16 * P)
    block_pos = []
    block_ch = []
    block_bi = []
    calls = []                                # (block0, nblocks, ch)
    groups = []                               # (pos0, npos)
    b = 0
    for g0 in range(0, TPC, GRP):
        gn = min(GRP, TPC - g0)
        groups.append((g0, gn))
        for ch in range(NCHUNK):
            nbk = int(NB[g0:g0 + gn, ch].sum())
            sub = []
            o = 0
            while o < nbk:
                n1 = min(MAXBLK, nbk - o)
                sub.append((b + o, n1))
                o += n1
            calls.append((sub, ch))
            for pos in range(g0, g0 + gn):
                for bi in range(int(NB[pos, ch])):
                    block_pos.append(pos)
                    block_ch.append(ch)
                    block_bi.append(bi)
                    b += 1
    assert b == B_TOTAL

    # per-core slot arrays
    blk_of = {}
    for bb in range(B_TOTAL):
        blk_of.setdefault((block_pos[bb], block_ch[bb]), []).append(bb)

    idx16 = np.zeros((NCORES, SLOTS), np.int16)
    meta = np.zeros((NCORES, P, 4 * B_TOTAL), np.float32)
    meta[:, :, 0::4] = -1.0                   # ld_lo
    meta[:, :, 2::4] = -1.0                   # ld_hi
    first_blk = {k: v[0] for k, v in blk_of.items()}
    for c in range(NCORES):
        inv_pos = np.zeros(TPC, np.int64)
        inv_pos[tile_perm[c]] = np.arange(TPC)
        for ch in range(NCHUNK):
            st = cores[c][ch]
            nslot = len(st["srow"])
            if nslot == 0:
                continue
            # order: by cell, paired slots first within cell
            ordr = np.lexsort((np.arange(nslot), ~st["spair"], st["scell"]))
            cell_s = st["scell"][ordr]
            pair_s = st["spair"][ordr]
            # global slot position for each ordered slot
            cb = np.searchsorted(cell_s, np.arange(TPC + 1))
            within = np.arange(nslot) - cb[cell_s]
            base_blk = np.array([first_blk[(int(inv_pos[tl]), ch)]
                                 for tl in range(TPC)], np.int64)
            npaired_cell = np.bincount(cell_s[pair_s], minlength=TPC)
            assert npaired_cell.max() <= P, npaired_cell.max()
            blk = base_blk[cell_s] + within // P
            prt = within % P
            slot = blk * P + prt
            idx16[c, slot] = st["srow"][ordr].astype(np.int16)
            meta[c, prt, 4 * blk] = st["ldlo"][ordr]
            meta[c, prt, 4 * blk + 1] = st["nrmlo"][ordr]
            meta[c, prt, 4 * blk + 2] = st["ldhi"][ordr]
            meta[c, prt, 4 * blk + 3] = st["nrmhi"][ordr]

    # wrapped idx layout [16, SLOTS/16] tiled to 128 partitions
    ii = np.arange(SLOTS)
    idxw = np.zeros((NCORES, 16, SLOTS // 16), np.int16)
    idxw[:, ii % 16, ii // 16] = idx16
    idxw = np.tile(idxw, (1, 8, 1))

    # node <-> (core, pos, ldcol) output mapping
    pos_of_tile = np.zeros(T_TILES, np.int64)
    for c in range(NCORES):
        pos_of_tile[tile_perm[c] + c * TPC] = np.arange(TPC)
    vnodes = np.arange(N)
    gi_core = assign[vnodes] // TPC
    gi_pos = pos_of_tile[assign[vnodes]]
    gi_j = ldcol[vnodes]
    # per-core pos-major node list [TPC, D] (entry: node id or N for pad)
    nodelist = np.full((NCORES, TPC, D), N, np.int64)
    for c in range(NCORES):
        tl = tile_perm[c] + c * TPC
        tn = tile_nodes[tl]                  # [TPC, D]
        nodelist[c] = np.where(tn >= 0, tn, N)

    return dict(cores=cores, NR=NR, NB=NB, B_TOTAL=B_TOTAL, SLOTS=SLOTS,
                calls=calls, groups=groups,
                block_pos=np.array(block_pos), block_ch=np.array(block_ch),
                block_bi=np.array(block_bi),
                idxw=idxw, meta=meta, idx16=idx16,
                gi_core=gi_core, gi_pos=gi_pos, gi_j=gi_j,
                nodelist=nodelist)


# ---------------------------------------------------------------------------
# device program builder
# ---------------------------------------------------------------------------

def _build_pass(g, mode, WROW, FP, HIOFF, Wo=None, KD=None, relu=False,
                out_f32=False):
    """One propagation pass.

    mode 'TP': psum acc [FP, D] (transposed); epilogue out = acc^T @ wcat
               (+ ones @ bcat) -> [D, Wo] tiles.
    mode 'CB': psum acc [D, FP]; epilogue acc += inT_tile^T @ wd
               + ident @ addin_tile; out = act(acc) -> [D, FP] tiles.
    """
    NR, NB, B_TOTAL, SLOTS = g["NR"], g["NB"], g["B_TOTAL"], g["SLOTS"]
    calls, groups = g["calls"], g["groups"]
    block_pos, block_ch, block_bi = g["block_pos"], g["block_ch"], g["block_bi"]

    nc = bacc.Bacc("TRN2", target_bir_lowering=False,
                   dynamic_dma_scratch_size=SCRATCH)
    tab = nc.declare_dram_parameter("tab", [NCHUNK * NR, WROW], BF16, isOutput=False)
    idx = nc.declare_dram_parameter("idx", [P, SLOTS // 16], I16, isOutput=False)
    meta = nc.declare_dram_parameter("meta", [P, 4 * B_TOTAL], F32, isOutput=False)
    iot = nc.declare_dram_parameter("iot", [P, D], BF16, isOutput=False)
    if mode == "TP":
        wcat = nc.declare_dram_parameter("wcat", [FP, Wo], BF16, isOutput=False)
        bcat = nc.declare_dram_parameter("bcat", [1, Wo], BF16, isOutput=False)
        out = nc.declare_dram_parameter("out", [D, TPC, Wo], BF16, isOutput=True)
        WO = Wo
    else:
        wd = nc.declare_dram_parameter("wd", [KD, FP], BF16, isOutput=False)
        inT = nc.declare_dram_parameter("inT", [KD, TPC * D], BF16, isOutput=False)
        addin = nc.declare_dram_parameter("addin", [D, TPC, FP], BF16, isOutput=False)
        ident = nc.declare_dram_parameter("ident", [D, D], BF16, isOutput=False)
        odt = F32 if out_f32 else BF16
        out = nc.declare_dram_parameter("out", [D, TPC, FP], odt, isOutput=True)
        WO = FP

    with ExitStack() as ctx:
        tc = ctx.enter_context(tile.TileContext(nc))
        cpool = ctx.enter_context(tc.tile_pool(name="const", bufs=1))
        gpool = ctx.enter_context(tc.tile_pool(name="g", bufs=11))
        spool = ctx.enter_context(tc.tile_pool(name="s", bufs=32))
        sbpool = ctx.enter_context(tc.tile_pool(name="sb", bufs=4))
        stpool = ctx.enter_context(tc.tile_pool(name="st", bufs=3))
        apool = ctx.enter_context(tc.tile_pool(name="acc", bufs=GRP, space="PSUM"))
        if mode == "TP":
            p2pool = ctx.enter_context(tc.tile_pool(name="p2", bufs=2, space="PSUM"))

        idx_t = cpool.tile([P, SLOTS // 16], I16)
        meta_t = cpool.tile([P, 4 * B_TOTAL], F32)
        # small leading piece first (first ~3 groups), bulk loads after --
        # lets the gather pipeline start ~5us earlier
        IH = min(((GRP * 2 * NCHUNK * 3) * 8 + 63) // 64 * 64, SLOTS // 16)
        MH = min((GRP * 2 * NCHUNK * 3) * 4, 4 * B_TOTAL)
        nc.sync.dma_start(out=idx_t[:, :IH], in_=idx[:, :IH])
        nc.sync.dma_start(out=meta_t[:, :MH], in_=meta[:, :MH])
        nc.sync.dma_start(out=idx_t[:, IH:], in_=idx[:, IH:])
        nc.sync.dma_start(out=meta_t[:, MH:], in_=meta[:, MH:])
        iota_b = cpool.tile([P, D], BF16)
        nc.sync.dma_start(out=iota_b[:], in_=iot[:])
        if mode == "TP":
            wcat_t = cpool.tile([FP, Wo], BF16)
            bcat_t = cpool.tile([1, Wo], BF16)
            ones_t = cpool.tile([1, D], BF16)
            nc.sync.dma_start(out=wcat_t[:], in_=wcat[:])
            nc.sync.dma_start(out=bcat_t[:], in_=bcat[:])
            nc.vector.memset(ones_t[:], 1.0)
        else:
            wd_t = cpool.tile([KD, FP], BF16)
            ident_t = cpool.tile([D, D], BF16)
            nc.sync.dma_start(out=wd_t[:], in_=wd[:])
            nc.sync.dma_start(out=ident_t[:], in_=ident[:])
            inpool = ctx.enter_context(tc.tile_pool(name="inp", bufs=3))
            adpool = ctx.enter_context(tc.tile_pool(name="adp", bufs=3))

        acc = {}
        ci = 0
        for (g0, gn) in groups:
            if mode == "CB":
                int_g = inpool.tile([KD, GRP * D], BF16, tag="inp")
                ad_g = adpool.tile([D, GRP, FP], BF16, tag="adp")
                nc.sync.dma_start(out=int_g[:, :gn * D],
                                  in_=inT[:, g0 * D:(g0 + gn) * D])
                nc.sync.dma_start(out=ad_g[:, :gn, :],
                                  in_=addin[:, g0:g0 + gn, :])
            for ch in range(NCHUNK):
                sub, _ch = calls[ci]
                ci += 1
                for (b0, nbk) in sub:
                    gt = gpool.tile([P, nbk, WROW], BF16, tag="g")
                    nc.gpsimd.dma_gather(
                        gt[:], tab[ch * NR:(ch + 1) * NR, :],
                        idx_t[:, b0 * 8:(b0 + nbk) * 8],
                        nbk * P, nbk * P, WROW,
                    )
                    for j in range(nbk):
                        bb = b0 + j
                    pos = int(block_pos[bb])
                    bi = int(block_bi[bb])
                    first = (ch == 0 and bi == 0)
                    last_s = (ch == NCHUNK - 1 and bi == int(NB[pos, ch]) - 1)
                    if first:
                        shape = [FP, D] if mode == "TP" else [D, FP]
                        acc[pos] = apool.tile(shape, F32, space="PSUM",
                                              tag="acc", name="acc")
                    mcol = 4 * bb
                    S_lo = spool.tile([P, D], BF16, tag="S")
                    nc.vector.tensor_scalar(
                        out=S_lo[:], in0=iota_b[:],
                        scalar1=meta_t[:, mcol:mcol + 1],
                        scalar2=meta_t[:, mcol + 1:mcol + 2],
                        op0=mybir.AluOpType.is_equal,
                        op1=mybir.AluOpType.mult,
                    )
                    # hi selector only on block 0 (paired slots live there)
                    do_hi = (bi == 0)
                    if mode == "TP":
                        nc.tensor.matmul(out=acc[pos][:], lhsT=gt[:, j, 0:FP],
                                         rhs=S_lo[:], start=first,
                                         stop=(mode == "TP" and last_s and not do_hi))
                    else:
                        nc.tensor.matmul(out=acc[pos][:], lhsT=S_lo[:],
                                         rhs=gt[:, j, 0:FP], start=first,
                                         stop=False)
                    if do_hi:
                        S_hi = spool.tile([P, D], BF16, tag="S")
                        nc.vector.tensor_scalar(
                            out=S_hi[:], in0=iota_b[:],
                            scalar1=meta_t[:, mcol + 2:mcol + 3],
                            scalar2=meta_t[:, mcol + 3:mcol + 4],
                            op0=mybir.AluOpType.is_equal,
                            op1=mybir.AluOpType.mult,
                        )
                        if mode == "TP":
                            nc.tensor.matmul(out=acc[pos][:],
                                             lhsT=gt[:, j, HIOFF:HIOFF + FP],
                                             rhs=S_hi[:], start=False,
                                             stop=last_s)
                        else:
                            nc.tensor.matmul(out=acc[pos][:], lhsT=S_hi[:],
                                             rhs=gt[:, j, HIOFF:HIOFF + FP],
                                             start=False, stop=False)
            # group epilogue
            st = stpool.tile([D, GRP, WO], F32 if (mode == "CB" and out_f32) else BF16, tag="st")
            for k in range(gn):
                pos = g0 + k
                if mode == "TP":
                    sb = sbpool.tile([FP, D], BF16, tag="sb")
                    nc.scalar.activation(sb[:], acc[pos][:],
                                         mybir.ActivationFunctionType.Copy)
                    p2 = p2pool.tile([D, Wo], F32, space="PSUM", tag="p2")
                    nc.tensor.matmul(out=p2[:], lhsT=sb[:], rhs=wcat_t[:],
                                     start=True, stop=False)
                    nc.tensor.matmul(out=p2[:], lhsT=ones_t[:], rhs=bcat_t[:],
                                     start=False, stop=True)
                    nc.scalar.activation(st[:, k, :], p2[:],
                                         mybir.ActivationFunctionType.Copy)
                else:
                    nc.tensor.matmul(out=acc[pos][:],
                                     lhsT=int_g[:, k * D:(k + 1) * D],
                                     rhs=wd_t[:], start=False, stop=False)
                    nc.tensor.matmul(out=acc[pos][:], lhsT=ident_t[:],
                                     rhs=ad_g[:, k, :],
                                     start=False, stop=True)
                    fn = (mybir.ActivationFunctionType.Relu if relu
                          else mybir.ActivationFunctionType.Copy)
                    nc.scalar.activation(st[:, k, :], acc[pos][:], fn)
                del acc[pos]
            nc.sync.dma_start(out=out[:, g0:g0 + gn, :], in_=st[:, :gn, :])

    nc.compile()
    return nc


# ---------------------------------------------------------------------------
# host glue
# ---------------------------------------------------------------------------

def _to_bf(a):
    return np.asarray(a, np.float32).astype(BF)


def _build_tables(g, feat, wrow, fp, hioff):
    """Per-core gather tables [NCHUNK*NR, wrow] from full features
    feat [N, fw] (fw = feat width <= fp slots in the row)."""
    NR = g["NR"]
    fw = feat.shape[1]
    featp = np.zeros((NCHUNK * CH, fw), BF)
    featp[:N] = feat
    tabs = []
    for c in range(NCORES):
        t = np.zeros((NCHUNK * NR, wrow), BF)
        for ch in range(NCHUNK):
            pairs = g["cores"][c][ch]["pairs"]
            nr = len(pairs)
            if nr == 0:
                continue
            base = featp[ch * CH:(ch + 1) * CH]
            t[ch * NR:ch * NR + nr, 0:fw] = base[pairs[:, 0]]
            t[ch * NR:ch * NR + nr, hioff:hioff + fw] = base[pairs[:, 1]]
        tabs.append(t)
    return tabs


def _unpermute(g, outs, fw):
    """outs: list of 8 arrays [D, TPC, >=fw] -> full [N, fw] float32."""
    stack = np.stack([np.asarray(o)[:, :, :fw] for o in outs])  # [C, D, TPC, fw]
    return stack[g["gi_core"], g["gi_j"], g["gi_pos"], :].astype(np.float32)


def _run(nc, in_maps):
    res = run_bass_kernel_spmd(nc, in_maps, list(range(NCORES)))
    return res.results


class _Programs:
    def __init__(self, g):
        self.g = g
        self.p1 = _build_pass(g, "TP", WROW=2 * F_IN, FP=F_IN, HIOFF=F_IN,
                              Wo=2 * F_HID)
        self.p2 = _build_pass(g, "CB", WROW=2 * F_HID, FP=F_HID, HIOFF=F_HID,
                              KD=F_IN, relu=True)
        self.p3 = _build_pass(g, "TP", WROW=2 * F_HID, FP=F_HID, HIOFF=F_HID,
                              Wo=2 * F_OUT)
        self.p4 = _build_pass(g, "CB", WROW=2 * F_HID, FP=F_OUT, HIOFF=F_OUT,
                              KD=F_HID, relu=False, out_f32=True)


def kernel(x, edge_index, edge_weight, W1, b1, W2, b2):
    x = np.asarray(x, np.float32)
    edge_index = np.asarray(edge_index)
    edge_weight = np.asarray(edge_weight, np.float32)
    W1 = np.asarray(W1, np.float32)
    b1 = np.asarray(b1, np.float32)
    W2 = np.asarray(W2, np.float32)
    b2 = np.asarray(b2, np.float32)

    g = _prep_graph(edge_index, edge_weight)
    progs = _Programs(g)
    return _run_all(g, progs, x, W1, b1, W2, b2)


_IOTA = np.tile(np.arange(D, dtype=np.float32).astype(BF)[None, :], (P, 1))


def _core_inputs(g, c, tabs, extra):
    m = {"tab": tabs[c], "idx": g["idxw"][c], "meta": g["meta"][c],
         "iot": _IOTA}
    m.update(extra(c) if callable(extra) else extra)
    return m


def _run_all(g, progs, x, W1, b1, W2, b2):
    xb = _to_bf(x)
    nodelist = g["nodelist"]                    # [C, TPC, D] node or N
    xpad = np.zeros((N + 1, F_IN), BF)
    xpad[:N] = xb

    # P1: propagate x; per tile emit [y1 | t11] = agg^T [2W12 | W11] + [0|b1]
    w1cat = np.concatenate([2.0 * W1[2], W1[1]], axis=1)        # [128, 128]
    b1cat = np.concatenate([np.zeros(F_HID, np.float32), b1])[None, :]
    tabs = _build_tables(g, xb, 2 * F_IN, F_IN, F_IN)
    maps = [_core_inputs(g, c, tabs,
                         {"wcat": w1cat.astype(BF), "bcat": b1cat.astype(BF)})
            for c in range(NCORES)]
    res = _run(progs.p1, maps)
    out1 = [np.asarray(r["out"]) for r in res]          # [D, TPC, 128] bf16
    y1_full = _unpermute(g, out1, F_HID).astype(BF)     # 2(Lx)W12 rows

    # P2: h = relu(L y1 + x (W10-W12) + t11)
    w2d = (W1[0] - W1[2]).astype(BF)                     # [128, 64]
    idn = np.eye(D, dtype=np.float32).astype(BF)
    tabs = _build_tables(g, y1_full, 2 * F_HID, F_HID, F_HID)

    def p2_extra(c):
        xt = xpad[nodelist[c].reshape(-1)]               # [TPC*D, 128]
        return {"wd": w2d,
                "inT": np.ascontiguousarray(xt.T).astype(BF),
                "addin": np.ascontiguousarray(
                    out1[c][:, :, F_HID:]).astype(BF),
                "ident": idn}
    maps = [_core_inputs(g, c, tabs, p2_extra) for c in range(NCORES)]
    res = _run(progs.p2, maps)
    outh = [np.asarray(r["out"]) for r in res]           # [D, TPC, 64] bf16
    h_full = _unpermute(g, outh, F_HID).astype(BF)

    # P3: propagate h; per tile emit [y2 | u21b]
    w2cat = np.concatenate([2.0 * W2[2], W2[1]], axis=1)         # [64, 80]
    b2cat = np.concatenate([np.zeros(F_OUT, np.float32), b2])[None, :]
    tabs = _build_tables(g, h_full, 2 * F_HID, F_HID, F_HID)
    maps = [_core_inputs(g, c, tabs,
                         {"wcat": w2cat.astype(BF), "bcat": b2cat.astype(BF)})
            for c in range(NCORES)]
    res = _run(progs.p3, maps)
    out3 = [np.asarray(r["out"]) for r in res]           # [D, TPC, 80] bf16
    y2_full = _unpermute(g, out3, F_OUT).astype(BF)

    # P4: out = L y2 + h (W20-W22) + u21b
    w4d = (W2[0] - W2[2]).astype(BF)                     # [64, 40]
    hpad = np.zeros((N + 1, F_HID), BF)
    hpad[:N] = h_full
    tabs = _build_tables(g, y2_full, 2 * F_HID, F_OUT, F_OUT)

    def p4_extra(c):
        ht = hpad[nodelist[c].reshape(-1)]
        return {"wd": w4d,
                "inT": np.ascontiguousarray(ht.T).astype(BF),
                "addin": np.ascontiguousarray(
                    out3[c][:, :, F_OUT:]).astype(BF),
                "ident": idn}
    maps = [_core_inputs(g, c, tabs, p4_extra) for c in range(NCORES)]
    res = _run(progs.p4, maps)
    out4 = [np.asarray(r["out"]) for r in res]           # [D, TPC, 40] f32
    return _unpermute(g, out4, F_OUT)
